# revision 49
# baseline (speedup 1.0000x reference)
"""Trainium2 Bass kernel for nn_MultiHeadAttention (B=2, S=4096, D=512, H=8).

Computes: q/k/v = relu(x@W+b) per head, softmax(q k^T / sqrt(64)) v,
out = relu(concat_heads @ Wo + bo).

Sharding: 8 cores = 2 (batch) x 4 (query-slice).  Each core computes full
K/V projections for its batch (redundant across the 4 q-slice cores) and
attention + output projection for its 1024-row query slice.  No collectives;
the host concatenates the 8 output slices.

Host-side prep: x is transposed feature-major per batch and shipped twice
(bf16 stile-major for K/Q projections, fp8 e4m3 for the V projection);
Wq/Wk arrive bf16 in output-tile(j)-major layout so the startup loads only
the pair-0 slices first; Wv is fp8, Wo bf16.

The kernel is softmax-exp-bound: 33.6M exps/core at ~1 elem/lane/cycle on
the 1.2 GHz ACT engine.  The design splits the exp stream across TWO
engines and keeps everything else off their critical paths:
  - scores^T = K^T_h.T @ Q^T_h per (head, ktile) in bf16: K=64 contraction,
    head pairs in different PE row-groups (concurrent), both heads of a
    ktile in ONE psS tile so the pair is released atomically by one exp op.
  - exp: pT = exp(s/8 - 2) in fp8 e4m3 (shift keeps values in e4m3 range;
    softmax is shift-invariant).  ~20/32 ktiles per block on ACT (exact exp,
    fused scale+bias, fp8 out); ~12/32 on DVE via the Schraudolph bit trick
    (int8 round of A*s + B bit-cast as e4m3 — one tensor_scalar per tile).
    Scores >= 0 (relu'd q,k) make the trick's domain safe; its ~4% sawtooth
    error cancels in the softmax ratio (measured end-to-end ~3e-3).
  - exps of two consecutive ktiles fill one [P, 2(kt), 2(head), QC] pT tile
    — exactly the operand the fp8 DoubleRow U matmul consumes: one DR MM
    per (ktile-pair, head) contracts both ktiles at once (~1.9x PE).  V_pad
    (fp8, per-head width padded to 66 for the DR 16B-stride rule) carries a
    ones column so U row 64 accumulates the softmax denominator for free.
    U matmuls trail their pair's exps by one group so the in-order PE never
    idles behind an in-flight exp.
  - normalize (2-block lag, hidden in ACT-bound stretches): denominator
    rows DMA-gathered [1,512]->[128,4] so ONE partition-parallel
    reciprocal_approx_fast covers a block (the [1,N] form is both slow and
    broken), scatter back, one gpsimd broadcast, DVE multiply into O^T.
  - projections ride as fillers inside attention blocks; V-proj uses fp8
    DoubleRow (x8 @ Wv8, quantization noise averages out over the K=512
    contraction and the attention sum); K/Q/O projections stay bf16 — fp8
    there amplifies through exp (tested: 6x worse error) and starving the
    PE triggers the HAM clock-gate, slowing every matmul.
  - startup: DMAs issued in exact consumption order on TWO hardware queues
    (Sync: q-path; ACT: k-path), tiny leading chunks ([0],[1],[2,3]) to
    start the first exps early, plus dummy K=1 warmup matmuls during the
    DMA wait to release the HAM clock-gate.
  - tail: two output-projection chains opened early (bias+3 pairs) during
    the last normalize, relus alternate ACT/DVE, per-row-tile DMA out.
"""

import numpy as np
import ml_dtypes

import concourse.bass as bass
import concourse.mybir as mybir
import concourse.tile as tile
from concourse import bacc
from concourse import bass_utils

F32 = mybir.dt.float32
BF16 = mybir.dt.bfloat16
FP8 = mybir.dt.float8e4
I8 = mybir.dt.int8
AF = mybir.ActivationFunctionType
ALU = mybir.AluOpType
DR = mybir.MatmulPerfMode.DoubleRow

P = 128
D = 512
H = 8
DH = 64
DT = D // P  # 4 (also = number of head pairs)
B = 2
S = 4096
NCORES = 8
QSPLIT = 4
SQ_FULL = S // QSPLIT  # 1024 query rows per core
QC = 512               # q-chunk (matmul free dim / PSUM bank width)

# ---- exp offload + fp8 attention weights.
# pT = exp(s/8 - 2) stored in fp8 e4m3 (the -2 shift keeps values in
# [e^-2, ~e^4.6], inside e4m3 range; softmax is shift-invariant since the
# denominator uses the same shifted exps).  Two producers:
#   ACT: exact exp via activation(scale=1/8, bias=-2) with fp8 output.
#   DVE (a fraction of groups, to unload the ACT bottleneck): Schraudolph bit
#     trick — e4m3(exp(s/8-2)) bit pattern ~= int8 round of A8*s + B8.
#     Scores are >= 0 (relu'd q,k) and bounded (~53 pre-scale), so the affine
#     stays in [32, ~110] (NaN at 120+), and the ~4% sawtooth error cancels
#     in the softmax ratio (validated <6e-3 end-to-end incl. fp8 V).
C0_SHIFT = 2.0
EXP8_A = 8.0 * 0.125 * 1.4426950408889634
EXP8_B = 56.0 - 8.0 * C0_SHIFT * 1.4426950408889634 - 0.35
# which exp ktiles of each steady attention block go to DVE (kt % 8)
DVE_RES = (1, 3, 6)


def build_mha(sk=S, sq=SQ_FULL, skip_vbias=False):
    """Build the SPMD Bass program (identical on all cores).

    All inputs arrive pre-tiled by the host into exact SBUF layout
    ([128 partitions, contiguous free bytes]) so every load is a max-packet
    linear DMA."""
    nc = bacc.Bacc("TRN2", target_bir_lowering=False, debug=False,
                   num_devices=NCORES)

    xT_d = nc.dram_tensor("xT_bf", (P, DT * sk), BF16,
                          kind="ExternalInput").ap()  # stile-major, see prep
    x8_d = nc.dram_tensor("xT_f8", (P, DT * sk), FP8,
                          kind="ExternalInput").ap()  # stile-major fp8 copy
    xqT_d = nc.dram_tensor("xqT_bf", (P, DT * sq), BF16,
                           kind="ExternalInput").ap()
    w_dram = {}
    for n in ("wq", "wk", "wo"):
        w_dram[n] = nc.dram_tensor(n, (P, DT * D), BF16,
                                   kind="ExternalInput").ap()
    w_dram["wv"] = nc.dram_tensor("wv", (P, DT * D), FP8,
                                  kind="ExternalInput").ap()
    b_dram = {
        "bq": nc.dram_tensor("bq", (P, DT), F32, kind="ExternalInput").ap(),
        "bk": nc.dram_tensor("bk", (P, DT), F32, kind="ExternalInput").ap(),
        "bv": nc.dram_tensor("bv", (1, D), FP8, kind="ExternalInput").ap(),
        "bo": nc.dram_tensor("bo", (1, D), BF16, kind="ExternalInput").ap(),
    }
    out = nc.dram_tensor("out", (sq, D), F32, kind="ExternalOutput").ap()

    with tile.TileContext(nc) as tc:
        _build_tile(tc, xT_d, x8_d, xqT_d, w_dram, b_dram, out, sk, sq,
                    skip_vbias)

    nc.compile()
    return nc


def _build_tile(tc, xT_d, x8_d, xqT_d, w_dram, b_dram, out, sk, sq,
                skip_vbias=False):
    nc = tc.nc
    SK_T = sk // P            # ktiles of the key/value sequence
    SQ_T = sq // P
    NQC = sq // QC            # q chunks per core
    CH = min(4, SK_T)         # stiles per projection chunk
    NCH = SK_T // CH
    KG = 1                    # ktiles per exp group

    with (
        tc.tile_pool(name="singles", bufs=1) as singles,
        tc.tile_pool(name="work", bufs=3) as work,
        tc.tile_pool(name="psum", bufs=2, space="PSUM") as psum,
    ):
        # ---- startup: DMAs issued in exactly the order the critical path
        # consumes them (queue executes in issue order).  wq/wk arrive in
        # j-major layout so the pair-0 slices are single contiguous DMAs;
        # x^T arrives stile-major so the first ktile is a small early DMA.
        w_bf = {}
        b_col = {}
        wq_src = w_dram["wq"].rearrange("p (j t c) -> p j t c", j=DT, t=DT)
        wk_src = w_dram["wk"].rearrange("p (j t c) -> p j t c", j=DT, t=DT)
        w_bf["wq"] = singles.tile([P, DT, DT, P], BF16, name="wq_bf")
        w_bf["wk"] = singles.tile([P, DT, DT, P], BF16, name="wk_bf")
        nc.sync.dma_start(w_bf["wq"][:, 0], wq_src[:, 0])
        b_col["bq"] = singles.tile([P, DT], F32, name="bq_col")
        nc.sync.dma_start(b_col["bq"], b_dram["bq"])
        xTq = singles.tile([P, DT, sq], BF16)
        xTq_src = xqT_d.rearrange("p (t s) -> p t s", t=DT)
        nc.sync.dma_start(xTq[:, :, 0:QC], xTq_src[:, :, 0:QC])
        # kproj-path inputs ride the ACT hardware DMA queue (idle until the
        # first exp) so they transfer in parallel with the qproj path above
        nc.scalar.dma_start(w_bf["wk"][:, 0], wk_src[:, 0])
        b_col["bk"] = singles.tile([P, DT], F32, name="bk_col")
        nc.scalar.dma_start(b_col["bk"], b_dram["bk"])
        xT = singles.tile([P, SK_T, DT, P], BF16)
        xT_src = xT_d.rearrange("p (s t c) -> p s t c", s=SK_T, t=DT)
        x8 = singles.tile([P, SK_T, DT, P], FP8)
        x8_src = x8_d.rearrange("p (s t c) -> p s t c", s=SK_T, t=DT)
        nc.scalar.dma_start(xT[:, 0:1], xT_src[:, 0:1])
        nc.scalar.dma_start(xT[:, 1:2], xT_src[:, 1:2])
        nc.scalar.dma_start(xT[:, 2:4], xT_src[:, 2:4])
        nc.scalar.dma_start(x8[:, 0:2], x8_src[:, 0:2])
        nc.scalar.dma_start(x8[:, 2:4], x8_src[:, 2:4])

        # ---- persistent SBUF tensors (memsets early: the warmup matmuls
        # below need xT1 before the input DMAs land) ----
        xT1 = singles.tile([1, sk], FP8)
        nc.vector.memset(xT1, 1.0)
        KT = singles.tile([P, DT, sk], BF16)
        # per-head width padded 65->66 so the DoubleRow weight-pair stride
        # (H*66 = 528 B) meets the 16B-alignment ISA restriction
        V_pad = singles.tile([P, SK_T, H, DH + 2], FP8)
        nc.vector.memset(V_pad[:, :, :, DH:DH + 1], 1.0)
        OT = singles.tile([P, DT, sq], BF16)
        OT1 = singles.tile([1, sq], BF16)
        nc.vector.memset(OT1, 1.0)
        negc0 = singles.tile([P, 1], F32, name="neg_c0")
        nc.vector.memset(negc0, -C0_SHIFT)
        ones64 = singles.tile([1, DH], F32, name="ones64")
        nc.vector.memset(ones64, 1.0)
        # dummy K=1 matmuls during the input-DMA wait: sustained PE activity
        # releases the HAM clock gate, so the startup projection chain and
        # early chunks run at full clock instead of 4/8
        for _ in range(8):
            psW = psum.tile([P, 2 * QC], F32, tag="scores", name="psS")
            nc.tensor.matmul(psW[:, 0:QC], xT1[0:1, 0:P], xT1[0:1, 0:QC],
                             start=True, stop=True)

        QT = singles.tile([P, DT, sq], BF16)

        def qproj(j, nq):
            psQ = psum.tile([P, QC], F32, tag="proj", name="psQ")
            for kt in range(DT):
                nc.tensor.matmul(
                    psQ, w_bf["wq"][:, j, kt, :],
                    xTq[:, kt, nq * QC:(nq + 1) * QC],
                    start=(kt == 0), stop=(kt == DT - 1))
            nc.vector.tensor_scalar(
                QT[:, j, nq * QC:(nq + 1) * QC], psQ,
                b_col["bq"][:, j:j + 1], 0.0, op0=ALU.add, op1=ALU.max)

        qproj(0, 0)

        # ---- bulk input DMAs, still roughly in consumption order ----
        b_row = {}
        if NQC > 1:
            nc.sync.dma_start(xTq[:, :, QC:sq], xTq_src[:, :, QC:sq])
            qproj(0, 1)
        wb = singles.tile([P, DT, D], FP8, name="wv_bf")
        nc.sync.dma_start(wb, w_dram["wv"].rearrange("p (t n) -> p t n",
                                                     t=DT))
        w_bf["wv"] = wb
        br = singles.tile([1, D], FP8, name="bv_row")
        nc.sync.dma_start(br, b_dram["bv"])
        b_row["bv"] = br
        for jj in range(1, DT):
            nc.sync.dma_start(w_bf["wk"][:, jj], wk_src[:, jj])
            nc.sync.dma_start(w_bf["wq"][:, jj], wq_src[:, jj])
        wb = singles.tile([P, DT, D], BF16, name="wo_bf")
        nc.sync.dma_start(wb, w_dram["wo"].rearrange("p (t n) -> p t n",
                                                     t=DT))
        w_bf["wo"] = wb
        br = singles.tile([1, D], BF16, name="bo_row")
        nc.sync.dma_start(br, b_dram["bo"])
        b_row["bo"] = br
        CHP = CH * P


        # PSUM tags: "proj" 2x1 banks, "scores" 1x4 banks, "psU" 2x1 = 8
        def vproj(st):
            psV = psum.tile([P, D], F32, tag="proj", name="psV")
            for tp in range(DT // 2):
                nc.tensor.matmul(
                    psV, x8[:, st, 2 * tp:2 * tp + 2, :],
                    w_bf["wv"][:, 2 * tp:2 * tp + 2, :],
                    start=(tp == 0),
                    stop=(skip_vbias and tp == DT // 2 - 1), perf_mode=DR)
            if not skip_vbias:
                nc.tensor.matmul(psV, xT1[:, st * P:(st + 1) * P],
                                 b_row["bv"], start=False, stop=True)
            nc.vector.tensor_scalar_max(
                V_pad[:, st, :, 0:DH],
                psV.rearrange("p (h d) -> p h d", h=H), 0.0)

        def kproj(j, st0, nst):
            psK = psum.tile([P, CHP], F32, tag="proj", name="psK")
            for kt in range(DT):
                nc.tensor.matmul(
                    psK[:, 0:nst * P], w_bf["wk"][:, j, kt, :],
                    xT[:, st0:st0 + nst, kt, :],
                    start=(kt == 0), stop=(kt == DT - 1))
            nc.vector.tensor_scalar(
                KT[:, j, st0 * P:(st0 + nst) * P], psK[:, 0:nst * P],
                b_col["bk"][:, j:j + 1], 0.0, op0=ALU.add, op1=ALU.max)

        NKTP = SK_T // 2  # ktile pairs per block

        def _exp_op(pT, psS, dve):
            if dve:
                nc.vector.tensor_scalar(
                    pT.bitcast(I8), psS, EXP8_A, EXP8_B,
                    op0=ALU.mult, op1=ALU.add)
            else:
                nc.scalar.activation(pT, psS, AF.Exp, bias=negc0,
                                     scale=0.125)

        def attn_kt_exp(j, qc, kt, pT_big, dve):
            """Scores (heads A||B, paired PE row groups, one shared psS tile
            so the pair is released atomically by ONE exp) + exp for ktile
            kt, written into plane kt%2 of pT_big."""
            q0 = qc * QC
            psS = psum.tile([P, 2 * QC], F32, tag="scores", bufs=2,
                            name="psS")
            for h in range(2):
                hp = h * DH
                nc.tensor.matmul(
                    psS[:, h * QC:(h + 1) * QC],
                    KT[hp:hp + DH, j, kt * P:(kt + 1) * P],
                    QT[hp:hp + DH, j, q0:q0 + QC], start=True, stop=True)
            _exp_op(pT_big[:, kt % 2], psS, dve)

        def attn_pair(j, qc, ktp, force_act=False, pt_tag="pT", pt_bufs=5):
            """One ktile pair: two scores+exp rounds into a shared
            [P, 2(kt), 2(h), QC] fp8 tile — the layout the DoubleRow U
            matmul consumes per head."""
            pT_big = work.tile([P, 2, 2, QC], FP8, tag=pt_tag, bufs=pt_bufs,
                               name=pt_tag)
            for i in range(2):
                kt = 2 * ktp + i
                dve = (not force_act) and (kt % 8) in DVE_RES
                attn_kt_exp(j, qc, kt, pT_big, dve)
            return pT_big

        def attn_u(j, ktp, pT_big, psU, h):
            """One DoubleRow fp8 matmul: psU[h] += V(kt).T P(kt) summed over
            the pair's 2 ktiles."""
            first, last = (ktp == 0), (ktp == NKTP - 1)
            nc.tensor.matmul(psU,
                             V_pad[:, 2 * ktp:2 * ktp + 2, 2 * j + h,
                                   0:DH + 1],
                             pT_big[:, :, h, :],
                             start=first, stop=last, perf_mode=DR)

        def u_pair(j, ktp, pT_big, psU):
            attn_u(j, ktp, pT_big, psU[0], 0)
            attn_u(j, ktp, pT_big, psU[1], 1)

        QCP = QC // P  # denom row [1, QC] reshapes to [P, QCP] for recip

        def attn_finish_copies(psU_A, psU_B):
            """Copy U out of PSUM fast — frees both accumulators for the
            next block.  Also DMA-gathers the two denominator rows into a
            [P, 2*QCP] collector so the reciprocal can run partition-parallel
            (a [1, QC] reciprocal serializes in one DVE lane).  Returns the
            SBUF copies and the collector."""
            ucs = []
            for psU in (psU_A, psU_B):
                uc = work.tile([DH + 1, QC], F32, tag="ucopy", bufs=5,
                               name="uc")
                nc.vector.tensor_copy(uc, psU)
                ucs.append(uc)
            den_t = work.tile([P, 2 * QCP], F32, tag="den", bufs=3,
                              name="den_t")
            for i, uc in enumerate(ucs):
                nc.sync.dma_start(den_t[:, i * QCP:(i + 1) * QCP],
                                  uc[DH:DH + 1, :])
            return ucs, den_t

        brc_sink = {}

        def normalize_thunks(j, qc, ucs, den_t, tail=False):
            """Per-head softmax normalize emitted later (as fillers inside
            the next block) so its latency hides under ACT-bound stretches.
            In tail mode the free-dim broadcast of the reciprocal row runs as
            a K=1 PE matmul into PSUM instead of the gpsimd partition
            broadcast — ~3us less latency on the final critical chain."""
            q0 = qc * QC
            rec_t = work.tile([P, 2 * QCP], F32, tag="rec", bufs=2,
                              name="rec_t")
            rrow2 = work.tile([1, 2 * QC], F32, tag="recip", bufs=3,
                              name="rrow2")
            brc2 = work.tile([DH, 2 * QC], F32, tag="brc", bufs=2,
                             name="brc2")

            def recip_bc():
                nc.vector.reciprocal_approx_fast(rec_t, den_t)
                for i in range(2):
                    nc.sync.dma_start(rrow2[:, i * QC:(i + 1) * QC],
                                      rec_t[:, i * QCP:(i + 1) * QCP])
                nc.gpsimd.partition_broadcast(brc2, rrow2)

            def one(uc, h0, i):
                def t():
                    nc.vector.tensor_mul(
                        OT[h0:h0 + DH, j, q0:q0 + QC], uc[0:DH, :],
                        brc2[:, i * QC:(i + 1) * QC])
                    brc_sink[(j, qc)] = brc2
                return t
            return [recip_bc, one(ucs[0], 0, 0), one(ucs[1], DH, 1)]

        def attn_span(j, qc, ktps, psU, fillers=(), precomputed=()):
            """Emit the exp groups of one attention block, sprinkling
            `fillers` (deferred work thunks) between groups so the in-order
            PE/DVE do them inside this ACT-bound stretch.  Returns this
            block's normalize thunks (to be run as fillers of the NEXT
            block)."""
            fillers = list(fillers)
            for pktp, ptile in precomputed:
                u_pair(j, pktp, ptile, psU)
            ngroups = 2 * len(ktps)
            spacing = max(1, ngroups // (len(fillers) + 1))
            gi = 0
            prev = None
            for ktp in ktps:
                # scores + exps for this pair first; the PREVIOUS pair's U
                # matmuls after, so the PE never sits in-queue behind an
                # in-flight exp (U(p) waits on exp(p); deferring it one pair
                # keeps the PE stream dependency-free)
                pTs = attn_pair(j, qc, ktp)
                if prev is not None:
                    u_pair(j, prev[0], prev[1], psU)
                prev = (ktp, pTs)
                for g in (gi + 1, gi + 2):
                    if fillers and g % spacing == 0:
                        fillers.pop(0)()
                gi += 2
            for f in fillers:
                f()
            u_pair(j, prev[0], prev[1], psU)
            if ktps[-1] == NKTP - 1:
                ucs, den_t = attn_finish_copies(psU[0], psU[1])
                tail = (j, qc) == (DT - 1, NQC - 1)
                return normalize_thunks(j, qc, ucs, den_t, tail=tail), ucs
            return [], None

        def new_psU():
            a = psum.tile([DH + 1, QC], F32, tag="psU", name="psU_A")
            b = psum.tile([DH + 1, QC], F32, tag="psU", name="psU_B")
            return (a, b)

        def outproj(qt):
            # bias matmul first: it reads OT1, whose re-write after the last
            # normalize acts as a scheduling gate for the whole chain (the
            # scheduler otherwise hoists these into mid-attention PE-idle
            # slots and stalls on under-modeled reciprocal latency)
            psO = psum.tile([P, D], F32, tag="proj", name="psO")
            nc.tensor.matmul(psO, OT1[:, qt * P:(qt + 1) * P],
                             b_row["bo"], start=True, stop=False)
            for j in range(DT):
                nc.tensor.matmul(psO, OT[:, j, qt * P:(qt + 1) * P],
                                 w_bf["wo"][:, j, :],
                                 start=False, stop=(j == DT - 1))
            o_sb = work.tile([P, D], F32, tag="osb", bufs=2, name="o_sb")
            if qt % 2 == 0 and qt >= SQ_T // NQC:
                # tail outprojs: half the relus on DVE so the two engines
                # drain the final chains in parallel
                nc.vector.tensor_scalar_max(o_sb, psO, 0.0)
            else:
                nc.scalar.activation(o_sb, psO, AF.Relu)
            nc.sync.dma_start(out[qt * P:(qt + 1) * P, :], o_sb)

        def gate_outproj(blk):
            """No-op rewrite of OT1 (max(1, recip<1) == 1) that depends on
            block `blk`'s normalize chain — gates the outproj chains (which
            start with an OT1-reading bias matmul) behind it, preventing the
            scheduler from hoisting them into mid-attention stalls."""
            brc = brc_sink[blk]
            nc.vector.tensor_scalar(OT1, OT1, brc[0:1, 0:1], None,
                                    op0=ALU.max)

        # ---- chunk loop: x load + V proj + K proj(pair 0) + attn(0, 0) ----
        psU0 = new_psU()
        N_STORE = 16
        store01 = []
        store01_kts = [0]
        store01_cur = [None]
        pendq = []   # queue of deferred normalize-thunk lists (2-block lag)
        # chunk structure: tiny leading chunks get the first exps going
        # ~10us earlier (stile-major x layout makes any stile range one
        # linear DMA); 4-stile chunks once the pipeline is primed
        CHUNKS = [[0], [1], [2, 3]] + [
            list(range(4 * k, 4 * k + 4)) for k in range(1, NCH)]
        cur00 = [None]
        for ci, ch in enumerate(CHUNKS):
            if ci >= 3:
                nc.sync.dma_start(xT[:, ch[0]:ch[-1] + 1],
                                  xT_src[:, ch[0]:ch[-1] + 1])
                nc.sync.dma_start(x8[:, ch[0]:ch[-1] + 1],
                                  x8_src[:, ch[0]:ch[-1] + 1])
            kproj(0, ch[0], len(ch))
            # QK + exp first: ACT can start before V exists (only U needs V)
            completed = []
            for kt in ch:
                if kt % 2 == 0:
                    cur00[0] = work.tile([P, 2, 2, QC], FP8, tag="pT",
                                         bufs=5, name="pT")
                dve00 = kt >= SK_T // 2 and (kt % 8) in DVE_RES
                attn_kt_exp(0, 0, kt, cur00[0], dve00)
                if kt % 2 == 1:
                    completed.append((kt // 2, cur00[0]))
            for st in ch:
                vproj(st)
            for ktp, pp in completed:
                u_pair(0, ktp, pp, psU0)
            # pre-compute exps of block (0,1) into held pT tiles: fills the
            # otherwise-idle ACT during the PE-bound chunk phase (the U
            # matmuls run later, so no PSUM cost).  Stored ktile idx <=
            # ch[-1], so its K tiles already exist; a pT01 tile is recorded
            # once both of its ktile planes are in.
            while (NQC > 1 and ci >= 3 and store01_kts[0] < N_STORE
                   and store01_kts[0] <= ch[-1]
                   and store01_kts[0] < (ci - 2) * N_STORE // (len(CHUNKS)
                                                               - 3) + 1):
                kt_s = store01_kts[0]
                if kt_s % 2 == 0:
                    store01_cur[0] = work.tile([P, 2, 2, QC], FP8,
                                               tag="pT01",
                                               bufs=N_STORE // 2,
                                               name="pT01")
                attn_kt_exp(0, 1, kt_s, store01_cur[0], False)
                if kt_s % 2 == 1:
                    store01.append((kt_s // 2, store01_cur[0]))
                store01_kts[0] += 1
            if ch[-1] == SK_T - 1:
                ucs0, den_t0 = attn_finish_copies(psU0[0], psU0[1])
                thunks = normalize_thunks(0, 0, ucs0, den_t0)
        pendq.append(thunks)

        # ---- remaining attention; fillers inside each ACT-bound block are:
        # the previous block's normalize chain + the next block's
        # projections (+ the qc0 half of the output projection during the
        # last block) ----
        blocks = [(0, qc) for qc in range(1, NQC)]
        blocks += [(j, qc) for j in range(1, DT) for qc in range(NQC)]
        owed = {blk: [] for blk in blocks}
        for (j, qc) in blocks:
            if (j, qc) != (0, 1):
                owed[(j, qc)].append(lambda j=j, qc=qc: qproj(j, qc))
            if qc == 0 and j >= 1:
                # 2-stile kproj fillers: a 4-stile one (4 MMs, ~1.7us) eats
                # more than the per-ktile PE slack and stalls the next
                # scores matmuls behind it in the in-order PE queue
                for n in range(2 * NCH):
                    owed[(j, qc)].append(lambda j=j, n=n: kproj(j, 2 * n, 2))
        for f in owed[blocks[0]]:
            f()
        for bi, (j, qc) in enumerate(blocks):
            # projection fillers first; normalize chains run with a 2-block
            # lag so their slow DVE reciprocals never sit near a block
            # boundary (where they would delay the relus feeding the next
            # pair's attention)
            fillers = []
            if bi + 1 < len(blocks):
                fillers += owed[blocks[bi + 1]]
            last = bi == len(blocks) - 1
            if last:
                # flush remaining normalize chains, then gate + emit the qc0
                # half of the output projection so it runs inside this block
                while pendq:
                    fillers += pendq.pop(0)
                if NQC > 1:
                    fillers += [lambda: gate_outproj((DT - 1, 0))]
                    fillers += [lambda qt=qt: outproj(qt)
                                for qt in range(SQ_T // NQC)]
            elif len(pendq) >= 2:
                fillers += pendq.pop(0)
                if bi == len(blocks) - 2 and pendq:
                    fillers += pendq.pop(0)
            psU = new_psU()
            if (j, qc) == (0, 1) and store01:
                thunks, ucs = attn_span(
                    j, qc, list(range(len(store01), NKTP)), psU,
                    fillers, precomputed=store01)
            else:
                thunks, ucs = attn_span(j, qc, list(range(NKTP)), psU,
                                        fillers)
            pendq.append(thunks)
            last_ucs = ucs

        # ---- tail: last block's normalize + remaining output rows ----
        # Two of the final outproj chains are gated only on the last block's
        # PSUM copies (their bias + pairs-0..2 matmuls need nothing newer),
        # so the PE does useful work during the slow reciprocal chain and
        # stays HAM-warm; their pair-3 matmul still waits on the real OT
        # write.  Gate writes go on DVE BEFORE the normalize thunks so they
        # are not queued behind the reciprocals.
        qt_lo = SQ_T // NQC if NQC > 1 else 0
        early = []
        open_psO = []
        if NQC > 1 and last_ucs is not None:
            early = [qt_lo, qt_lo + 1]
            for qt, uc in zip(early, last_ucs):
                nc.vector.tensor_scalar(
                    OT1[:, qt * P:(qt + 1) * P],
                    OT1[:, qt * P:(qt + 1) * P],
                    uc[DH:DH + 1, 0:1], None, op0=ALU.min)
            # partial chains (bias + pairs 0..2): no pair-3 matmul yet, so
            # the in-order PE runs all 8 matmuls during the reciprocals
            # instead of stalling at the first chain's pair-3 wait
            for qt in early:
                psO = psum.tile([P, D], F32, tag="proj", name="psO")
                nc.tensor.matmul(psO, OT1[:, qt * P:(qt + 1) * P],
                                 b_row["bo"], start=True, stop=False)
                for j in range(DT - 1):
                    nc.tensor.matmul(psO, OT[:, j, qt * P:(qt + 1) * P],
                                     w_bf["wo"][:, j, :],
                                     start=False, stop=False)
                open_psO.append((qt, psO))
        while pendq:
            for f in pendq.pop(0):
                f()
        for qt, psO in open_psO:
            nc.tensor.matmul(psO, OT[:, DT - 1, qt * P:(qt + 1) * P],
                             w_bf["wo"][:, DT - 1, :],
                             start=False, stop=True)
            o_sb = work.tile([P, D], F32, tag="osb", bufs=2, name="o_sb")
            if qt % 2 == 0:
                nc.vector.tensor_scalar_max(o_sb, psO, 0.0)
            else:
                nc.scalar.activation(o_sb, psO, AF.Relu)
            nc.sync.dma_start(out[qt * P:(qt + 1) * P, :], o_sb)
        gate_outproj(blocks[-1])
        for qt in range(qt_lo, SQ_T):
            if qt not in early:
                outproj(qt)


_NC_CACHE = {}


def _get_nc(sk=S, sq=SQ_FULL, skip_vbias=False):
    key = (sk, sq, skip_vbias)
    if key not in _NC_CACHE:
        _NC_CACHE[key] = build_mha(sk, sq, skip_vbias)
    return _NC_CACHE[key]


def _tile_rows(a):
    """[D, n] -> SBUF layout [P, DT*n]: partition p gets rows p, 128+p, ..."""
    Dd, n = a.shape
    t = Dd // P
    return np.ascontiguousarray(
        a.reshape(t, P, n).transpose(1, 0, 2).reshape(P, t * n))


def _tile_rows_j(a):
    """[D, D] -> SBUF layout [P, DT(j)*DT(t)*P]: output-tile-major so each
    128-col output slice (one head pair's weights) is one contiguous DMA."""
    Dd, n = a.shape
    t, nj = Dd // P, n // P
    return np.ascontiguousarray(
        a.reshape(t, P, nj, P).transpose(1, 2, 0, 3).reshape(P, -1))


def _tile_chunks(a, chp):
    """[D, sk] -> chunk-major SBUF layout [P, NCH*DT*chp]: per partition,
    sequence chunks outermost so each chunk is one contiguous linear DMA."""
    Dd, sk = a.shape
    t, nch = Dd // P, sk // chp
    return np.ascontiguousarray(
        a.reshape(t, P, nch, chp).transpose(1, 2, 0, 3).reshape(P, -1))


def prep_inputs(x, Wq, bq, Wk, bk, Wv, bv, Wo, bo):
    """Host-side sharding/layout prep: bf16 casts, feature-major transpose,
    SBUF pre-tiling.  Returns the 8 per-core input maps."""
    bf = ml_dtypes.bfloat16
    f8 = ml_dtypes.float8_e4m3
    x = np.asarray(x, dtype=np.float32)
    shared = {
        "wq": _tile_rows_j(np.asarray(Wq, np.float32).astype(bf)),
        "wk": _tile_rows_j(np.asarray(Wk, np.float32).astype(bf)),
        "wv": _tile_rows(np.asarray(Wv, np.float32).astype(f8)),
        "wo": _tile_rows(np.asarray(Wo, np.float32).astype(bf)),
        "bq": np.ascontiguousarray(
            np.asarray(bq, np.float32).reshape(DT, P).T),
        "bk": np.ascontiguousarray(
            np.asarray(bk, np.float32).reshape(DT, P).T),
        "bv": np.asarray(bv, np.float32).astype(f8).reshape(1, D),
        "bo": np.asarray(bo, np.float32).astype(bf).reshape(1, D),
    }
    xT_b = [x[b].T.astype(bf) for b in range(B)]
    in_maps = []
    for c in range(NCORES):
        b, qo = divmod(c, QSPLIT)
        m = dict(shared)
        m["xT_bf"] = _tile_chunks(xT_b[b], P)
        m["xT_f8"] = _tile_chunks(xT_b[b].astype(f8), P)
        m["xqT_bf"] = _tile_rows(
            xT_b[b][:, qo * SQ_FULL:(qo + 1) * SQ_FULL])
        in_maps.append(m)
    return in_maps


def kernel(x, Wq, bq, Wk, bk, Wv, bv, Wo, bo, **run_kwargs):
    """Full-input entry point: shards across 8 NeuronCores, returns full out."""
    in_maps = prep_inputs(x, Wq, bq, Wk, bk, Wv, bv, Wo, bo)
    nc = _get_nc(skip_vbias=bool(np.all(np.asarray(bv) == 0)))
    res = bass_utils.run_bass_kernel_spmd(
        nc, in_maps, core_ids=list(range(NCORES)), **run_kwargs)
    full = np.empty((B, S, D), np.float32)
    for c in range(NCORES):
        b, qo = divmod(c, QSPLIT)
        full[b, qo * SQ_FULL:(qo + 1) * SQ_FULL] = res.results[c]["out"]
    if run_kwargs:
        return full, res
    return full



# revision 50
# speedup vs baseline: 1.1783x; 1.1783x over previous
"""Trainium2 Bass kernel for nn_MultiHeadAttention (B=2, S=4096, D=512, H=8).

Computes: q/k/v = relu(x@W+b) per head, softmax(q k^T / sqrt(64)) v,
out = relu(concat_heads @ Wo + bo).

Sharding: 8 cores = 2 (batch) x 4 (query-slice).  Each core computes full
K/V projections for its batch (redundant across the 4 q-slice cores) and
attention + output projection for its 1024-row query slice.  No collectives;
the host concatenates the 8 output slices.

Host-side prep: x is transposed feature-major per batch and shipped twice
(bf16 stile-major for K/Q projections, fp8 e4m3 for the V projection);
Wq/Wk arrive bf16 in output-tile(j)-major layout so the startup loads only
the pair-0 slices first; Wv is fp8, Wo bf16.

The kernel is softmax-exp-bound: 33.6M exps/core at ~1 elem/lane/cycle on
the 1.2 GHz ACT engine.  The design splits the exp stream across TWO
engines and keeps everything else off their critical paths:
  - scores^T = K^T_h.T @ Q^T_h per (head, ktile) in bf16: K=64 contraction,
    head pairs in different PE row-groups (concurrent), both heads of a
    ktile in ONE psS tile so the pair is released atomically by one exp op.
  - exp: pT = exp(s/8 - 2) in fp8 e4m3 (shift keeps values in e4m3 range;
    softmax is shift-invariant).  ~20/32 ktiles per block on ACT (exact exp,
    fused scale+bias, fp8 out); ~12/32 on DVE via the Schraudolph bit trick
    (int8 round of A*s + B bit-cast as e4m3 — one tensor_scalar per tile).
    Scores >= 0 (relu'd q,k) make the trick's domain safe; its ~4% sawtooth
    error cancels in the softmax ratio (measured end-to-end ~3e-3).
  - exps of two consecutive ktiles fill one [P, 2(kt), 2(head), QC] pT tile
    — exactly the operand the fp8 DoubleRow U matmul consumes: one DR MM
    per (ktile-pair, head) contracts both ktiles at once (~1.9x PE).  V_pad
    (fp8, per-head width padded to 66 for the DR 16B-stride rule) carries a
    ones column so U row 64 accumulates the softmax denominator for free.
    U matmuls trail their pair's exps by one group so the in-order PE never
    idles behind an in-flight exp.
  - normalize (2-block lag, hidden in ACT-bound stretches): denominator
    rows DMA-gathered [1,512]->[128,4] so ONE partition-parallel
    reciprocal_approx_fast covers a block (the [1,N] form is both slow and
    broken), scatter back, one gpsimd broadcast, DVE multiply into O^T.
  - projections ride as fillers inside attention blocks; V-proj uses fp8
    DoubleRow (x8 @ Wv8, quantization noise averages out over the K=512
    contraction and the attention sum); K/Q/O projections stay bf16 — fp8
    there amplifies through exp (tested: 6x worse error) and starving the
    PE triggers the HAM clock-gate, slowing every matmul.
  - startup: DMAs issued in exact consumption order on TWO hardware queues
    (Sync: q-path; ACT: k-path), tiny leading chunks ([0],[1],[2,3]) to
    start the first exps early, plus dummy K=1 warmup matmuls during the
    DMA wait to release the HAM clock-gate.
  - tail: two output-projection chains opened early (bias+3 pairs) during
    the last normalize, relus alternate ACT/DVE, per-row-tile DMA out.
"""

import numpy as np
import ml_dtypes

import concourse.bass as bass
import concourse.mybir as mybir
import concourse.tile as tile
from concourse import bacc
from concourse import bass_utils

F32 = mybir.dt.float32
BF16 = mybir.dt.bfloat16
FP8 = mybir.dt.float8e4
I8 = mybir.dt.int8
AF = mybir.ActivationFunctionType
ALU = mybir.AluOpType
DR = mybir.MatmulPerfMode.DoubleRow

P = 128
D = 512
H = 8
DH = 64
DT = D // P  # 4 (also = number of head pairs)
B = 2
S = 4096
NCORES = 8
QSPLIT = 4
SQ_FULL = S // QSPLIT  # 1024 query rows per core
QC = 512               # q-chunk (matmul free dim / PSUM bank width)

# ---- exp offload + fp8 attention weights.
# pT = exp(s/8 - 2) stored in fp8 e4m3 (the -2 shift keeps values in
# [e^-2, ~e^4.6], inside e4m3 range; softmax is shift-invariant since the
# denominator uses the same shifted exps).  Two producers:
#   ACT: exact exp via activation(scale=1/8, bias=-2) with fp8 output.
#   DVE (a fraction of groups, to unload the ACT bottleneck): Schraudolph bit
#     trick — e4m3(exp(s/8-2)) bit pattern ~= int8 round of A8*s + B8.
#     Scores are >= 0 (relu'd q,k) and bounded (~53 pre-scale), so the affine
#     stays in [32, ~110] (NaN at 120+), and the ~4% sawtooth error cancels
#     in the softmax ratio (validated <6e-3 end-to-end incl. fp8 V).
C0_SHIFT = 2.0
EXP8_A = 8.0 * 0.125 * 1.4426950408889634
EXP8_B = 56.0 - 8.0 * C0_SHIFT * 1.4426950408889634 - 0.35
# which exp ktiles of each steady attention block go to DVE (kt % 8)
DVE_RES = (1, 3, 6)


def build_mha(sk=S, sq=SQ_FULL, skip_vbias=False):
    """Build the SPMD Bass program (identical on all cores).

    All inputs arrive pre-tiled by the host into exact SBUF layout
    ([128 partitions, contiguous free bytes]) so every load is a max-packet
    linear DMA."""
    nc = bacc.Bacc("TRN2", target_bir_lowering=False, debug=False,
                   num_devices=NCORES)

    xT_d = nc.dram_tensor("xT_bf", (P, DT * sk), BF16,
                          kind="ExternalInput").ap()  # stile-major, see prep
    x8_d = nc.dram_tensor("xT_f8", (P, DT * sk), FP8,
                          kind="ExternalInput").ap()  # stile-major fp8 copy
    xqT_d = nc.dram_tensor("xqT_bf", (P, DT * sq), BF16,
                           kind="ExternalInput").ap()
    w_dram = {}
    for n in ("wq", "wk", "wo"):
        w_dram[n] = nc.dram_tensor(n, (P, DT * D), BF16,
                                   kind="ExternalInput").ap()
    w_dram["wv"] = nc.dram_tensor("wv", (P, DT * D), FP8,
                                  kind="ExternalInput").ap()
    b_dram = {
        "bq": nc.dram_tensor("bq", (P, DT), F32, kind="ExternalInput").ap(),
        "bk": nc.dram_tensor("bk", (P, DT), F32, kind="ExternalInput").ap(),
        "bv": nc.dram_tensor("bv", (1, D), FP8, kind="ExternalInput").ap(),
        "bo": nc.dram_tensor("bo", (1, D), BF16, kind="ExternalInput").ap(),
    }
    out = nc.dram_tensor("out", (sq, D), F32, kind="ExternalOutput").ap()

    with tile.TileContext(nc) as tc:
        _build_tile(tc, xT_d, x8_d, xqT_d, w_dram, b_dram, out, sk, sq,
                    skip_vbias)

    nc.compile()
    return nc


def _build_tile(tc, xT_d, x8_d, xqT_d, w_dram, b_dram, out, sk, sq,
                skip_vbias=False):
    nc = tc.nc
    SK_T = sk // P            # ktiles of the key/value sequence
    SQ_T = sq // P
    NQC = sq // QC            # q chunks per core
    CH = min(4, SK_T)         # stiles per projection chunk
    NCH = SK_T // CH
    KG = 1                    # ktiles per exp group

    with (
        tc.tile_pool(name="singles", bufs=1) as singles,
        tc.tile_pool(name="work", bufs=3) as work,
        tc.tile_pool(name="psum", bufs=2, space="PSUM") as psum,
    ):
        # ---- startup: DMAs issued in exactly the order the critical path
        # consumes them (queue executes in issue order).  wq/wk arrive in
        # j-major layout so the pair-0 slices are single contiguous DMAs;
        # x^T arrives stile-major so the first ktile is a small early DMA.
        w_bf = {}
        b_col = {}
        wq_src = w_dram["wq"].rearrange("p (j t c) -> p j t c", j=DT, t=DT)
        wk_src = w_dram["wk"].rearrange("p (j t c) -> p j t c", j=DT, t=DT)
        w_bf["wq"] = singles.tile([P, DT, DT, P], BF16, name="wq_bf")
        w_bf["wk"] = singles.tile([P, DT, DT, P], BF16, name="wk_bf")
        nc.sync.dma_start(w_bf["wq"][:, 0], wq_src[:, 0])
        b_col["bq"] = singles.tile([P, DT], F32, name="bq_col")
        nc.sync.dma_start(b_col["bq"], b_dram["bq"])
        xTq = singles.tile([P, DT, sq], BF16)
        xTq_src = xqT_d.rearrange("p (t s) -> p t s", t=DT)
        nc.sync.dma_start(xTq[:, :, 0:QC], xTq_src[:, :, 0:QC])
        # kproj-path inputs ride the ACT hardware DMA queue (idle until the
        # first exp) so they transfer in parallel with the qproj path above
        nc.scalar.dma_start(w_bf["wk"][:, 0], wk_src[:, 0])
        b_col["bk"] = singles.tile([P, DT], F32, name="bk_col")
        nc.scalar.dma_start(b_col["bk"], b_dram["bk"])
        xT = singles.tile([P, SK_T, DT, P], BF16)
        xT_src = xT_d.rearrange("p (s t c) -> p s t c", s=SK_T, t=DT)
        x8 = singles.tile([P, SK_T, DT, P], FP8)
        x8_src = x8_d.rearrange("p (s t c) -> p s t c", s=SK_T, t=DT)
        nc.scalar.dma_start(xT[:, 0:1], xT_src[:, 0:1])
        nc.scalar.dma_start(xT[:, 1:2], xT_src[:, 1:2])
        nc.scalar.dma_start(xT[:, 2:4], xT_src[:, 2:4])
        nc.scalar.dma_start(x8[:, 0:2], x8_src[:, 0:2])
        nc.scalar.dma_start(x8[:, 2:4], x8_src[:, 2:4])

        # ---- persistent SBUF tensors (memsets early: the warmup matmuls
        # below need xT1 before the input DMAs land) ----
        xT1 = singles.tile([1, sk], FP8)
        nc.vector.memset(xT1, 1.0)
        KT = singles.tile([P, DT, sk], BF16)
        # per-head width padded 65->66 so the DoubleRow weight-pair stride
        # (H*66 = 528 B) meets the 16B-alignment ISA restriction
        V_pad = singles.tile([P, SK_T, H, DH + 2], FP8)
        nc.vector.memset(V_pad[:, :, :, DH:DH + 1], 1.0)
        OT = singles.tile([P, DT, sq], BF16)
        OT1 = singles.tile([1, sq], BF16)
        nc.vector.memset(OT1, 1.0)
        negc0 = singles.tile([P, 1], F32, name="neg_c0")
        nc.vector.memset(negc0, -C0_SHIFT)
        ones64 = singles.tile([1, DH], F32, name="ones64")
        nc.vector.memset(ones64, 1.0)
        # dummy K=1 matmuls during the input-DMA wait: sustained PE activity
        # releases the HAM clock gate, so the startup projection chain and
        # early chunks run at full clock instead of 4/8
        for _ in range(8):
            psW = psum.tile([P, 2 * QC], F32, tag="scores", name="psS")
            nc.tensor.matmul(psW[:, 0:QC], xT1[0:1, 0:P], xT1[0:1, 0:QC],
                             start=True, stop=True)

        QT = singles.tile([P, DT, sq], BF16)

        def qproj(j, nq):
            psQ = psum.tile([P, QC], F32, tag="proj", name="psQ")
            for kt in range(DT):
                nc.tensor.matmul(
                    psQ, w_bf["wq"][:, j, kt, :],
                    xTq[:, kt, nq * QC:(nq + 1) * QC],
                    start=(kt == 0), stop=(kt == DT - 1))
            nc.vector.tensor_scalar(
                QT[:, j, nq * QC:(nq + 1) * QC], psQ,
                b_col["bq"][:, j:j + 1], 0.0, op0=ALU.add, op1=ALU.max)

        qproj(0, 0)

        # ---- bulk input DMAs, still roughly in consumption order ----
        b_row = {}
        if NQC > 1:
            nc.sync.dma_start(xTq[:, :, QC:sq], xTq_src[:, :, QC:sq])
            qproj(0, 1)
        wb = singles.tile([P, DT, D], FP8, name="wv_bf")
        nc.sync.dma_start(wb, w_dram["wv"].rearrange("p (t n) -> p t n",
                                                     t=DT))
        w_bf["wv"] = wb
        br = singles.tile([1, D], FP8, name="bv_row")
        nc.sync.dma_start(br, b_dram["bv"])
        b_row["bv"] = br
        for jj in range(1, DT):
            nc.sync.dma_start(w_bf["wk"][:, jj], wk_src[:, jj])
            nc.sync.dma_start(w_bf["wq"][:, jj], wq_src[:, jj])
        wb = singles.tile([P, DT, D], BF16, name="wo_bf")
        nc.sync.dma_start(wb, w_dram["wo"].rearrange("p (t n) -> p t n",
                                                     t=DT))
        w_bf["wo"] = wb
        br = singles.tile([1, D], BF16, name="bo_row")
        nc.sync.dma_start(br, b_dram["bo"])
        b_row["bo"] = br
        CHP = CH * P


        # PSUM tags: "proj" 2x1 banks, "scores" 1x4 banks, "psU" 2x1 = 8
        def vproj(st):
            psV = psum.tile([P, D], F32, tag="proj", name="psV")
            for tp in range(DT // 2):
                nc.tensor.matmul(
                    psV, x8[:, st, 2 * tp:2 * tp + 2, :],
                    w_bf["wv"][:, 2 * tp:2 * tp + 2, :],
                    start=(tp == 0),
                    stop=(skip_vbias and tp == DT // 2 - 1), perf_mode=DR)
            if not skip_vbias:
                nc.tensor.matmul(psV, xT1[:, st * P:(st + 1) * P],
                                 b_row["bv"], start=False, stop=True)
            nc.vector.tensor_scalar_max(
                V_pad[:, st, :, 0:DH],
                psV.rearrange("p (h d) -> p h d", h=H), 0.0)

        def kproj(j, st0, nst):
            psK = psum.tile([P, CHP], F32, tag="proj", name="psK")
            for kt in range(DT):
                nc.tensor.matmul(
                    psK[:, 0:nst * P], w_bf["wk"][:, j, kt, :],
                    xT[:, st0:st0 + nst, kt, :],
                    start=(kt == 0), stop=(kt == DT - 1))
            nc.vector.tensor_scalar(
                KT[:, j, st0 * P:(st0 + nst) * P], psK[:, 0:nst * P],
                b_col["bk"][:, j:j + 1], 0.0, op0=ALU.add, op1=ALU.max)

        NKTP = SK_T // 2  # ktile pairs per block

        def _exp_op(pT, psS, dve):
            if dve:
                nc.vector.tensor_scalar(
                    pT.bitcast(I8), psS, EXP8_A, EXP8_B,
                    op0=ALU.mult, op1=ALU.add)
            else:
                nc.scalar.activation(pT, psS, AF.Exp, bias=negc0,
                                     scale=0.125)

        def attn_kt_exp(j, qc, kt, pT_big, dve):
            """Scores (heads A||B, paired PE row groups, one shared psS tile
            so the pair is released atomically by ONE exp) + exp for ktile
            kt, written into plane kt%2 of pT_big."""
            q0 = qc * QC
            psS = psum.tile([P, 2 * QC], F32, tag="scores", bufs=2,
                            name="psS")
            for h in range(2):
                hp = h * DH
                nc.tensor.matmul(
                    psS[:, h * QC:(h + 1) * QC],
                    KT[hp:hp + DH, j, kt * P:(kt + 1) * P],
                    QT[hp:hp + DH, j, q0:q0 + QC], start=True, stop=True)
            _exp_op(pT_big[:, kt % 2], psS, dve)

        def attn_pair(j, qc, ktp, force_act=False, pt_tag="pT", pt_bufs=5):
            """One ktile pair: two scores+exp rounds into a shared
            [P, 2(kt), 2(h), QC] fp8 tile — the layout the DoubleRow U
            matmul consumes per head."""
            pT_big = work.tile([P, 2, 2, QC], FP8, tag=pt_tag, bufs=pt_bufs,
                               name=pt_tag)
            for i in range(2):
                kt = 2 * ktp + i
                dve = (not force_act) and (kt % 8) in DVE_RES
                attn_kt_exp(j, qc, kt, pT_big, dve)
            return pT_big

        def attn_u(j, ktp, pT_big, psU, h):
            """One DoubleRow fp8 matmul: psU[h] += V(kt).T P(kt) summed over
            the pair's 2 ktiles."""
            first, last = (ktp == 0), (ktp == NKTP - 1)
            nc.tensor.matmul(psU,
                             V_pad[:, 2 * ktp:2 * ktp + 2, 2 * j + h,
                                   0:DH + 1],
                             pT_big[:, :, h, :],
                             start=first, stop=last, perf_mode=DR)

        def u_pair(j, ktp, pT_big, psU):
            attn_u(j, ktp, pT_big, psU[0], 0)
            attn_u(j, ktp, pT_big, psU[1], 1)

        QCP = QC // P  # denom row [1, QC] reshapes to [P, QCP] for recip

        def attn_finish_copies(psU_A, psU_B):
            """Copy U out of PSUM fast — frees both accumulators for the
            next block.  Also DMA-gathers the two denominator rows into a
            [P, 2*QCP] collector so the reciprocal can run partition-parallel
            (a [1, QC] reciprocal serializes in one DVE lane).  Returns the
            SBUF copies and the collector."""
            ucs = []
            for psU in (psU_A, psU_B):
                uc = work.tile([DH + 1, QC], F32, tag="ucopy", bufs=5,
                               name="uc")
                nc.vector.tensor_copy(uc, psU)
                ucs.append(uc)
            den_t = work.tile([P, 2 * QCP], F32, tag="den", bufs=3,
                              name="den_t")
            for i, uc in enumerate(ucs):
                nc.sync.dma_start(den_t[:, i * QCP:(i + 1) * QCP],
                                  uc[DH:DH + 1, :])
            return ucs, den_t

        brc_sink = {}

        def normalize_thunks(j, qc, ucs, den_t, tail=False):
            """Per-head softmax normalize emitted later (as fillers inside
            the next block) so its latency hides under ACT-bound stretches.
            In tail mode the free-dim broadcast of the reciprocal row runs as
            a K=1 PE matmul into PSUM instead of the gpsimd partition
            broadcast — ~3us less latency on the final critical chain."""
            q0 = qc * QC
            rec_t = work.tile([P, 2 * QCP], F32, tag="rec", bufs=2,
                              name="rec_t")
            rrow2 = work.tile([1, 2 * QC], F32, tag="recip", bufs=3,
                              name="rrow2")
            brc2 = work.tile([DH, 2 * QC], F32, tag="brc", bufs=2,
                             name="brc2")

            def recip_bc():
                nc.vector.reciprocal_approx_fast(rec_t, den_t)
                for i in range(2):
                    nc.sync.dma_start(rrow2[:, i * QC:(i + 1) * QC],
                                      rec_t[:, i * QCP:(i + 1) * QCP])
                nc.gpsimd.partition_broadcast(brc2, rrow2)

            def one(uc, h0, i):
                def t():
                    nc.vector.tensor_mul(
                        OT[h0:h0 + DH, j, q0:q0 + QC], uc[0:DH, :],
                        brc2[:, i * QC:(i + 1) * QC])
                    brc_sink[(j, qc)] = brc2
                return t
            return [recip_bc, one(ucs[0], 0, 0), one(ucs[1], DH, 1)]

        def attn_span(j, qc, ktps, psU, fillers=(), precomputed=()):
            """Emit the exp groups of one attention block, sprinkling
            `fillers` (deferred work thunks) between groups so the in-order
            PE/DVE do them inside this ACT-bound stretch.  Returns this
            block's normalize thunks (to be run as fillers of the NEXT
            block)."""
            fillers = list(fillers)
            for pktp, ptile in precomputed:
                u_pair(j, pktp, ptile, psU)
            ngroups = 2 * len(ktps)
            spacing = max(1, ngroups // (len(fillers) + 1))
            gi = 0
            prev = None
            for ktp in ktps:
                # scores + exps for this pair first; the PREVIOUS pair's U
                # matmuls after, so the PE never sits in-queue behind an
                # in-flight exp (U(p) waits on exp(p); deferring it one pair
                # keeps the PE stream dependency-free)
                pTs = attn_pair(j, qc, ktp)
                if prev is not None:
                    u_pair(j, prev[0], prev[1], psU)
                prev = (ktp, pTs)
                for g in (gi + 1, gi + 2):
                    if fillers and g % spacing == 0:
                        fillers.pop(0)()
                gi += 2
            for f in fillers:
                f()
            u_pair(j, prev[0], prev[1], psU)
            if ktps[-1] == NKTP - 1:
                ucs, den_t = attn_finish_copies(psU[0], psU[1])
                tail = (j, qc) == (DT - 1, NQC - 1)
                return normalize_thunks(j, qc, ucs, den_t, tail=tail), ucs
            return [], None

        def new_psU():
            a = psum.tile([DH + 1, QC], F32, tag="psU", name="psU_A")
            b = psum.tile([DH + 1, QC], F32, tag="psU", name="psU_B")
            return (a, b)

        def outproj(qt):
            # bias matmul first: it reads OT1, whose re-write after the last
            # normalize acts as a scheduling gate for the whole chain (the
            # scheduler otherwise hoists these into mid-attention PE-idle
            # slots and stalls on under-modeled reciprocal latency)
            psO = psum.tile([P, D], F32, tag="proj", name="psO")
            nc.tensor.matmul(psO, OT1[:, qt * P:(qt + 1) * P],
                             b_row["bo"], start=True, stop=False)
            for j in range(DT):
                nc.tensor.matmul(psO, OT[:, j, qt * P:(qt + 1) * P],
                                 w_bf["wo"][:, j, :],
                                 start=False, stop=(j == DT - 1))
            o_sb = work.tile([P, D], F32, tag="osb", bufs=2, name="o_sb")
            if qt % 2 == 0 and qt >= SQ_T // NQC:
                # tail outprojs: half the relus on DVE so the two engines
                # drain the final chains in parallel
                nc.vector.tensor_scalar_max(o_sb, psO, 0.0)
            else:
                nc.scalar.activation(o_sb, psO, AF.Relu)
            nc.sync.dma_start(out[qt * P:(qt + 1) * P, :], o_sb)

        def gate_outproj(blk):
            """No-op rewrite of OT1 (max(1, recip<1) == 1) that depends on
            block `blk`'s normalize chain — gates the outproj chains (which
            start with an OT1-reading bias matmul) behind it, preventing the
            scheduler from hoisting them into mid-attention stalls."""
            brc = brc_sink[blk]
            nc.vector.tensor_scalar(OT1, OT1, brc[0:1, 0:1], None,
                                    op0=ALU.max)

        # ---- chunk loop: x load + V proj + K proj(pair 0) + attn(0, 0) ----
        psU0 = new_psU()
        N_STORE = 16
        store01 = []
        store01_kts = [0]
        store01_cur = [None]
        pendq = []   # queue of deferred normalize-thunk lists (2-block lag)
        # chunk structure: tiny leading chunks get the first exps going
        # ~10us earlier (stile-major x layout makes any stile range one
        # linear DMA); 4-stile chunks once the pipeline is primed
        CHUNKS = [[0], [1], [2, 3]] + [
            list(range(4 * k, 4 * k + 4)) for k in range(1, NCH)]
        cur00 = [None]
        for ci, ch in enumerate(CHUNKS):
            if ci >= 3:
                nc.sync.dma_start(xT[:, ch[0]:ch[-1] + 1],
                                  xT_src[:, ch[0]:ch[-1] + 1])
                nc.sync.dma_start(x8[:, ch[0]:ch[-1] + 1],
                                  x8_src[:, ch[0]:ch[-1] + 1])
            kproj(0, ch[0], len(ch))
            # QK + exp first: ACT can start before V exists (only U needs V)
            completed = []
            for kt in ch:
                if kt % 2 == 0:
                    cur00[0] = work.tile([P, 2, 2, QC], FP8, tag="pT",
                                         bufs=5, name="pT")
                dve00 = kt >= SK_T // 2 and (kt % 8) in DVE_RES
                attn_kt_exp(0, 0, kt, cur00[0], dve00)
                if kt % 2 == 1:
                    completed.append((kt // 2, cur00[0]))
            for st in ch:
                vproj(st)
            for ktp, pp in completed:
                u_pair(0, ktp, pp, psU0)
            # pre-compute exps of block (0,1) into held pT tiles: fills the
            # otherwise-idle ACT during the PE-bound chunk phase (the U
            # matmuls run later, so no PSUM cost).  Stored ktile idx <=
            # ch[-1], so its K tiles already exist; a pT01 tile is recorded
            # once both of its ktile planes are in.
            while (NQC > 1 and ci >= 3 and store01_kts[0] < N_STORE
                   and store01_kts[0] <= ch[-1]
                   and store01_kts[0] < (ci - 2) * N_STORE // (len(CHUNKS)
                                                               - 3) + 1):
                kt_s = store01_kts[0]
                if kt_s % 2 == 0:
                    store01_cur[0] = work.tile([P, 2, 2, QC], FP8,
                                               tag="pT01",
                                               bufs=N_STORE // 2,
                                               name="pT01")
                attn_kt_exp(0, 1, kt_s, store01_cur[0], False)
                if kt_s % 2 == 1:
                    store01.append((kt_s // 2, store01_cur[0]))
                store01_kts[0] += 1
            if ch[-1] == SK_T - 1:
                ucs0, den_t0 = attn_finish_copies(psU0[0], psU0[1])
                thunks = normalize_thunks(0, 0, ucs0, den_t0)
        pendq.append(thunks)

        # ---- remaining attention; fillers inside each ACT-bound block are:
        # the previous block's normalize chain + the next block's
        # projections (+ the qc0 half of the output projection during the
        # last block) ----
        blocks = [(0, qc) for qc in range(1, NQC)]
        blocks += [(j, qc) for j in range(1, DT) for qc in range(NQC)]
        owed = {blk: [] for blk in blocks}
        for (j, qc) in blocks:
            if (j, qc) != (0, 1):
                owed[(j, qc)].append(lambda j=j, qc=qc: qproj(j, qc))
            if qc == 0 and j >= 1:
                # early-half K tiles as fillers of the PREVIOUS block
                for n in range(NCH // 2):
                    owed[(j, qc)].append(lambda j=j, n=n: kproj(j, 4 * n, 4))
            if qc == 1:
                # late-half K tiles of the NEXT pair run inside block (j,0)
                # itself (emitted well before their ktiles are consumed) —
                # halves the filler load of the j-change blocks, which
                # otherwise stalls the scores matmuls behind ~15us of
                # filler work in the in-order PE queue
                if j + 1 < DT:
                    for n in range(NCH // 2, NCH):
                        owed[(j, qc)].insert(
                            0, lambda j2=j + 1, n=n: kproj(j2, 4 * n, 4))
        for f in owed[blocks[0]]:
            f()
        for bi, (j, qc) in enumerate(blocks):
            # projection fillers first; normalize chains run with a 2-block
            # lag so their slow DVE reciprocals never sit near a block
            # boundary (where they would delay the relus feeding the next
            # pair's attention)
            fillers = []
            if bi + 1 < len(blocks):
                fillers += owed[blocks[bi + 1]]
            last = bi == len(blocks) - 1
            if last:
                # flush remaining normalize chains, then gate + emit the qc0
                # half of the output projection so it runs inside this block
                while pendq:
                    fillers += pendq.pop(0)
                if NQC > 1:
                    fillers += [lambda: gate_outproj((DT - 1, 0))]
                    fillers += [lambda qt=qt: outproj(qt)
                                for qt in range(SQ_T // NQC)]
            elif len(pendq) >= 2:
                fillers += pendq.pop(0)
                if bi == len(blocks) - 2 and pendq:
                    fillers += pendq.pop(0)
            psU = new_psU()
            if (j, qc) == (0, 1) and store01:
                thunks, ucs = attn_span(
                    j, qc, list(range(len(store01), NKTP)), psU,
                    fillers, precomputed=store01)
            else:
                thunks, ucs = attn_span(j, qc, list(range(NKTP)), psU,
                                        fillers)
            pendq.append(thunks)
            last_ucs = ucs

        # ---- tail: last block's normalize + remaining output rows ----
        # Two of the final outproj chains are gated only on the last block's
        # PSUM copies (their bias + pairs-0..2 matmuls need nothing newer),
        # so the PE does useful work during the slow reciprocal chain and
        # stays HAM-warm; their pair-3 matmul still waits on the real OT
        # write.  Gate writes go on DVE BEFORE the normalize thunks so they
        # are not queued behind the reciprocals.
        qt_lo = SQ_T // NQC if NQC > 1 else 0
        early = []
        open_psO = []
        if NQC > 1 and last_ucs is not None:
            early = [qt_lo, qt_lo + 1]
            for qt, uc in zip(early, last_ucs):
                nc.vector.tensor_scalar(
                    OT1[:, qt * P:(qt + 1) * P],
                    OT1[:, qt * P:(qt + 1) * P],
                    uc[DH:DH + 1, 0:1], None, op0=ALU.min)
            # partial chains (bias + pairs 0..2): no pair-3 matmul yet, so
            # the in-order PE runs all 8 matmuls during the reciprocals
            # instead of stalling at the first chain's pair-3 wait
            for qt in early:
                psO = psum.tile([P, D], F32, tag="proj", name="psO")
                nc.tensor.matmul(psO, OT1[:, qt * P:(qt + 1) * P],
                                 b_row["bo"], start=True, stop=False)
                for j in range(DT - 1):
                    nc.tensor.matmul(psO, OT[:, j, qt * P:(qt + 1) * P],
                                     w_bf["wo"][:, j, :],
                                     start=False, stop=False)
                open_psO.append((qt, psO))
        while pendq:
            for f in pendq.pop(0):
                f()
        for qt, psO in open_psO:
            nc.tensor.matmul(psO, OT[:, DT - 1, qt * P:(qt + 1) * P],
                             w_bf["wo"][:, DT - 1, :],
                             start=False, stop=True)
            o_sb = work.tile([P, D], F32, tag="osb", bufs=2, name="o_sb")
            if qt % 2 == 0:
                nc.vector.tensor_scalar_max(o_sb, psO, 0.0)
            else:
                nc.scalar.activation(o_sb, psO, AF.Relu)
            nc.sync.dma_start(out[qt * P:(qt + 1) * P, :], o_sb)
        gate_outproj(blocks[-1])
        for qt in range(qt_lo, SQ_T):
            if qt not in early:
                outproj(qt)


_NC_CACHE = {}


def _get_nc(sk=S, sq=SQ_FULL, skip_vbias=False):
    key = (sk, sq, skip_vbias)
    if key not in _NC_CACHE:
        _NC_CACHE[key] = build_mha(sk, sq, skip_vbias)
    return _NC_CACHE[key]


def _tile_rows(a):
    """[D, n] -> SBUF layout [P, DT*n]: partition p gets rows p, 128+p, ..."""
    Dd, n = a.shape
    t = Dd // P
    return np.ascontiguousarray(
        a.reshape(t, P, n).transpose(1, 0, 2).reshape(P, t * n))


def _tile_rows_j(a):
    """[D, D] -> SBUF layout [P, DT(j)*DT(t)*P]: output-tile-major so each
    128-col output slice (one head pair's weights) is one contiguous DMA."""
    Dd, n = a.shape
    t, nj = Dd // P, n // P
    return np.ascontiguousarray(
        a.reshape(t, P, nj, P).transpose(1, 2, 0, 3).reshape(P, -1))


def _tile_chunks(a, chp):
    """[D, sk] -> chunk-major SBUF layout [P, NCH*DT*chp]: per partition,
    sequence chunks outermost so each chunk is one contiguous linear DMA."""
    Dd, sk = a.shape
    t, nch = Dd // P, sk // chp
    return np.ascontiguousarray(
        a.reshape(t, P, nch, chp).transpose(1, 2, 0, 3).reshape(P, -1))


def prep_inputs(x, Wq, bq, Wk, bk, Wv, bv, Wo, bo):
    """Host-side sharding/layout prep: bf16 casts, feature-major transpose,
    SBUF pre-tiling.  Returns the 8 per-core input maps."""
    bf = ml_dtypes.bfloat16
    f8 = ml_dtypes.float8_e4m3
    x = np.asarray(x, dtype=np.float32)
    shared = {
        "wq": _tile_rows_j(np.asarray(Wq, np.float32).astype(bf)),
        "wk": _tile_rows_j(np.asarray(Wk, np.float32).astype(bf)),
        "wv": _tile_rows(np.asarray(Wv, np.float32).astype(f8)),
        "wo": _tile_rows(np.asarray(Wo, np.float32).astype(bf)),
        "bq": np.ascontiguousarray(
            np.asarray(bq, np.float32).reshape(DT, P).T),
        "bk": np.ascontiguousarray(
            np.asarray(bk, np.float32).reshape(DT, P).T),
        "bv": np.asarray(bv, np.float32).astype(f8).reshape(1, D),
        "bo": np.asarray(bo, np.float32).astype(bf).reshape(1, D),
    }
    xT_b = [x[b].T.astype(bf) for b in range(B)]
    in_maps = []
    for c in range(NCORES):
        b, qo = divmod(c, QSPLIT)
        m = dict(shared)
        m["xT_bf"] = _tile_chunks(xT_b[b], P)
        m["xT_f8"] = _tile_chunks(xT_b[b].astype(f8), P)
        m["xqT_bf"] = _tile_rows(
            xT_b[b][:, qo * SQ_FULL:(qo + 1) * SQ_FULL])
        in_maps.append(m)
    return in_maps


def kernel(x, Wq, bq, Wk, bk, Wv, bv, Wo, bo, **run_kwargs):
    """Full-input entry point: shards across 8 NeuronCores, returns full out."""
    in_maps = prep_inputs(x, Wq, bq, Wk, bk, Wv, bv, Wo, bo)
    nc = _get_nc(skip_vbias=bool(np.all(np.asarray(bv) == 0)))
    res = bass_utils.run_bass_kernel_spmd(
        nc, in_maps, core_ids=list(range(NCORES)), **run_kwargs)
    full = np.empty((B, S, D), np.float32)
    for c in range(NCORES):
        b, qo = divmod(c, QSPLIT)
        full[b, qo * SQ_FULL:(qo + 1) * SQ_FULL] = res.results[c]["out"]
    if run_kwargs:
        return full, res
    return full



# revision 51
# speedup vs baseline: 1.1836x; 1.0045x over previous
"""Trainium2 Bass kernel for nn_MultiHeadAttention (B=2, S=4096, D=512, H=8).

Computes: q/k/v = relu(x@W+b) per head, softmax(q k^T / sqrt(64)) v,
out = relu(concat_heads @ Wo + bo).

Sharding: 8 cores = 2 (batch) x 4 (query-slice).  Each core computes full
K/V projections for its batch (redundant across the 4 q-slice cores) and
attention + output projection for its 1024-row query slice.  No collectives;
the host concatenates the 8 output slices.

Host-side prep (part of the sharding/layout step, not device compute):
x is cast to bf16 and transposed to feature-major x^T per batch, and the
weight matrices are cast to bf16 — the tensor engine contracts along the
partition dim, so all device matmuls consume feature-major operands.

Per-core kernel (all matmuls bf16 with fp32 PSUM accumulation):
  - K^T, Q^T computed feature-major: lhsT=W tile, rhs=x^T.  Bias+relu fused
    on DVE (bias is per-partition in this layout).
  - V computed in natural [s, d] layout (lhsT = x^T tile, rhs = Wv); bias via
    a K=1 ones-row matmul; relu on DVE; stored per head with a ones column
    appended (V_pad) so the attention U matmul also produces the softmax
    denominator row for free.
  - scores^T = K^T_h.T @ Q^T_h per (head, ktile): K=64 contraction; heads are
    processed in pairs at base partitions 0/64 so the two matmuls run
    concurrently in different PE row-groups.
  - exp on ACT (scale=1/8 fused), no max-subtraction (relu'd q/k make scores
    bounded: measured range [0, 6.6]).  ACT exp is the kernel's throughput
    floor (~1 elem/lane/cycle): exp ops span 2 ktiles x 2 heads (4 PSUM
    banks) to amortize the per-op overhead, the first attention block is
    interleaved with the K/V projection chunks, and the remaining
    projections are emitted between attention blocks so the PE does them
    inside ACT-bound stretches.
  - U^T[65, q] = V_pad_h.T @ P^T accumulated over ktiles in PSUM; row 64 is
    the denominator.  U^T is copied to SBUF immediately (releases the PSUM
    accumulator for the next block), then normalized off the critical path:
    DVE reciprocal + gpsimd partition broadcast + DVE multiply into
    feature-major O^T.
  - out = relu(O^T.T @ Wo + bo) via lhsT=O^T tiles, rhs=Wo; bias via ones-row
    matmul; relu on ACT; DMA to HBM.
"""

import numpy as np
import ml_dtypes

import concourse.bass as bass
import concourse.mybir as mybir
import concourse.tile as tile
from concourse import bacc
from concourse import bass_utils

F32 = mybir.dt.float32
BF16 = mybir.dt.bfloat16
FP8 = mybir.dt.float8e4
I8 = mybir.dt.int8
AF = mybir.ActivationFunctionType
ALU = mybir.AluOpType
DR = mybir.MatmulPerfMode.DoubleRow

P = 128
D = 512
H = 8
DH = 64
DT = D // P  # 4 (also = number of head pairs)
B = 2
S = 4096
NCORES = 8
QSPLIT = 4
SQ_FULL = S // QSPLIT  # 1024 query rows per core
QC = 512               # q-chunk (matmul free dim / PSUM bank width)

# ---- exp offload + fp8 attention weights.
# pT = exp(s/8 - 2) stored in fp8 e4m3 (the -2 shift keeps values in
# [e^-2, ~e^4.6], inside e4m3 range; softmax is shift-invariant since the
# denominator uses the same shifted exps).  Two producers:
#   ACT: exact exp via activation(scale=1/8, bias=-2) with fp8 output.
#   DVE (a fraction of groups, to unload the ACT bottleneck): Schraudolph bit
#     trick — e4m3(exp(s/8-2)) bit pattern ~= int8 round of A8*s + B8.
#     Scores are >= 0 (relu'd q,k) and bounded (~53 pre-scale), so the affine
#     stays in [32, ~110] (NaN at 120+), and the ~4% sawtooth error cancels
#     in the softmax ratio (validated <6e-3 end-to-end incl. fp8 V).
C0_SHIFT = 2.0
EXP8_A = 8.0 * 0.125 * 1.4426950408889634
EXP8_B = 56.0 - 8.0 * C0_SHIFT * 1.4426950408889634 - 0.35
# which exp ktiles of each steady attention block go to DVE (kt % 8)
DVE_RES = (1, 3, 6)


def build_mha(sk=S, sq=SQ_FULL, skip_vbias=False):
    """Build the SPMD Bass program (identical on all cores).

    All inputs arrive pre-tiled by the host into exact SBUF layout
    ([128 partitions, contiguous free bytes]) so every load is a max-packet
    linear DMA."""
    nc = bacc.Bacc("TRN2", target_bir_lowering=False, debug=False,
                   num_devices=NCORES)

    xT_d = nc.dram_tensor("xT_bf", (P, DT * sk), BF16,
                          kind="ExternalInput").ap()  # stile-major, see prep
    x8_d = nc.dram_tensor("xT_f8", (P, DT * sk), FP8,
                          kind="ExternalInput").ap()  # stile-major fp8 copy
    xqT_d = nc.dram_tensor("xqT_bf", (P, DT * sq), BF16,
                           kind="ExternalInput").ap()
    w_dram = {}
    for n in ("wq", "wk", "wo"):
        w_dram[n] = nc.dram_tensor(n, (P, DT * D), BF16,
                                   kind="ExternalInput").ap()
    w_dram["wv"] = nc.dram_tensor("wv", (P, DT * D), FP8,
                                  kind="ExternalInput").ap()
    b_dram = {
        "bq": nc.dram_tensor("bq", (P, DT), F32, kind="ExternalInput").ap(),
        "bk": nc.dram_tensor("bk", (P, DT), F32, kind="ExternalInput").ap(),
        "bv": nc.dram_tensor("bv", (1, D), FP8, kind="ExternalInput").ap(),
        "bo": nc.dram_tensor("bo", (1, D), BF16, kind="ExternalInput").ap(),
    }
    out = nc.dram_tensor("out", (sq, D), F32, kind="ExternalOutput").ap()

    with tile.TileContext(nc) as tc:
        _build_tile(tc, xT_d, x8_d, xqT_d, w_dram, b_dram, out, sk, sq,
                    skip_vbias)

    nc.compile()
    return nc


def _build_tile(tc, xT_d, x8_d, xqT_d, w_dram, b_dram, out, sk, sq,
                skip_vbias=False):
    nc = tc.nc
    SK_T = sk // P            # ktiles of the key/value sequence
    SQ_T = sq // P
    NQC = sq // QC            # q chunks per core
    CH = min(4, SK_T)         # stiles per projection chunk
    NCH = SK_T // CH
    KG = 1                    # ktiles per exp group

    with (
        tc.tile_pool(name="singles", bufs=1) as singles,
        tc.tile_pool(name="work", bufs=3) as work,
        tc.tile_pool(name="psum", bufs=2, space="PSUM") as psum,
    ):
        # ---- startup: DMAs issued in exactly the order the critical path
        # consumes them (queue executes in issue order).  wq/wk arrive in
        # j-major layout so the pair-0 slices are single contiguous DMAs;
        # x^T arrives stile-major so the first ktile is a small early DMA.
        w_bf = {}
        b_col = {}
        wq_src = w_dram["wq"].rearrange("p (j t c) -> p j t c", j=DT, t=DT)
        wk_src = w_dram["wk"].rearrange("p (j t c) -> p j t c", j=DT, t=DT)
        w_bf["wq"] = singles.tile([P, DT, DT, P], BF16, name="wq_bf")
        w_bf["wk"] = singles.tile([P, DT, DT, P], BF16, name="wk_bf")
        nc.sync.dma_start(w_bf["wq"][:, 0], wq_src[:, 0])
        b_col["bq"] = singles.tile([P, DT], F32, name="bq_col")
        nc.sync.dma_start(b_col["bq"], b_dram["bq"])
        xTq = singles.tile([P, DT, sq], BF16)
        xTq_src = xqT_d.rearrange("p (t s) -> p t s", t=DT)
        nc.sync.dma_start(xTq[:, :, 0:QC], xTq_src[:, :, 0:QC])
        # kproj-path inputs ride the ACT hardware DMA queue (idle until the
        # first exp) so they transfer in parallel with the qproj path above
        nc.scalar.dma_start(w_bf["wk"][:, 0], wk_src[:, 0])
        b_col["bk"] = singles.tile([P, DT], F32, name="bk_col")
        nc.scalar.dma_start(b_col["bk"], b_dram["bk"])
        xT = singles.tile([P, SK_T, DT, P], BF16)
        xT_src = xT_d.rearrange("p (s t c) -> p s t c", s=SK_T, t=DT)
        x8 = singles.tile([P, SK_T, DT, P], FP8)
        x8_src = x8_d.rearrange("p (s t c) -> p s t c", s=SK_T, t=DT)
        nc.scalar.dma_start(xT[:, 0:1], xT_src[:, 0:1])
        nc.scalar.dma_start(xT[:, 1:2], xT_src[:, 1:2])
        nc.scalar.dma_start(xT[:, 2:4], xT_src[:, 2:4])
        nc.scalar.dma_start(x8[:, 0:2], x8_src[:, 0:2])
        nc.scalar.dma_start(x8[:, 2:4], x8_src[:, 2:4])

        # ---- persistent SBUF tensors (memsets early: the warmup matmuls
        # below need xT1 before the input DMAs land) ----
        xT1 = singles.tile([1, sk], FP8)
        nc.vector.memset(xT1, 1.0)
        KT = singles.tile([P, DT, sk], BF16)
        # per-head width padded 65->66 so the DoubleRow weight-pair stride
        # (H*66 = 528 B) meets the 16B-alignment ISA restriction
        V_pad = singles.tile([P, SK_T, H, DH + 2], FP8)
        nc.vector.memset(V_pad[:, :, :, DH:DH + 1], 1.0)
        OT = singles.tile([P, DT, sq], BF16)
        OT1 = singles.tile([1, sq], BF16)
        nc.vector.memset(OT1, 1.0)
        negc0 = singles.tile([P, 1], F32, name="neg_c0")
        nc.vector.memset(negc0, -C0_SHIFT)
        ones64 = singles.tile([1, DH], F32, name="ones64")
        nc.vector.memset(ones64, 1.0)
        # dummy K=1 matmuls during the input-DMA wait: sustained PE activity
        # releases the HAM clock gate, so the startup projection chain and
        # early chunks run at full clock instead of 4/8
        for _ in range(8):
            psW = psum.tile([P, 2 * QC], F32, tag="scores", name="psS")
            nc.tensor.matmul(psW[:, 0:QC], xT1[0:1, 0:P], xT1[0:1, 0:QC],
                             start=True, stop=True)

        QT = singles.tile([P, DT, sq], BF16)

        def qproj(j, nq):
            psQ = psum.tile([P, QC], F32, tag="proj", name="psQ")
            for kt in range(DT):
                nc.tensor.matmul(
                    psQ, w_bf["wq"][:, j, kt, :],
                    xTq[:, kt, nq * QC:(nq + 1) * QC],
                    start=(kt == 0), stop=(kt == DT - 1))
            nc.vector.tensor_scalar(
                QT[:, j, nq * QC:(nq + 1) * QC], psQ,
                b_col["bq"][:, j:j + 1], 0.0, op0=ALU.add, op1=ALU.max)

        qproj(0, 0)

        # ---- bulk input DMAs, still roughly in consumption order ----
        b_row = {}
        if NQC > 1:
            nc.sync.dma_start(xTq[:, :, QC:sq], xTq_src[:, :, QC:sq])
            qproj(0, 1)
        wb = singles.tile([P, DT, D], FP8, name="wv_bf")
        nc.sync.dma_start(wb, w_dram["wv"].rearrange("p (t n) -> p t n",
                                                     t=DT))
        w_bf["wv"] = wb
        br = singles.tile([1, D], FP8, name="bv_row")
        nc.sync.dma_start(br, b_dram["bv"])
        b_row["bv"] = br
        for jj in range(1, DT):
            nc.sync.dma_start(w_bf["wk"][:, jj], wk_src[:, jj])
            nc.sync.dma_start(w_bf["wq"][:, jj], wq_src[:, jj])
        wb = singles.tile([P, DT, D], BF16, name="wo_bf")
        nc.sync.dma_start(wb, w_dram["wo"].rearrange("p (t n) -> p t n",
                                                     t=DT))
        w_bf["wo"] = wb
        br = singles.tile([1, D], BF16, name="bo_row")
        nc.sync.dma_start(br, b_dram["bo"])
        b_row["bo"] = br
        CHP = CH * P


        # PSUM tags: "proj" 2x1 banks, "scores" 1x4 banks, "psU" 2x1 = 8
        def vproj(st):
            psV = psum.tile([P, D], F32, tag="proj", name="psV")
            for tp in range(DT // 2):
                nc.tensor.matmul(
                    psV, x8[:, st, 2 * tp:2 * tp + 2, :],
                    w_bf["wv"][:, 2 * tp:2 * tp + 2, :],
                    start=(tp == 0),
                    stop=(skip_vbias and tp == DT // 2 - 1), perf_mode=DR)
            if not skip_vbias:
                nc.tensor.matmul(psV, xT1[:, st * P:(st + 1) * P],
                                 b_row["bv"], start=False, stop=True)
            nc.vector.tensor_scalar_max(
                V_pad[:, st, :, 0:DH],
                psV.rearrange("p (h d) -> p h d", h=H), 0.0)

        def kproj(j, st0, nst):
            psK = psum.tile([P, CHP], F32, tag="proj", name="psK")
            for kt in range(DT):
                nc.tensor.matmul(
                    psK[:, 0:nst * P], w_bf["wk"][:, j, kt, :],
                    xT[:, st0:st0 + nst, kt, :],
                    start=(kt == 0), stop=(kt == DT - 1))
            nc.vector.tensor_scalar(
                KT[:, j, st0 * P:(st0 + nst) * P], psK[:, 0:nst * P],
                b_col["bk"][:, j:j + 1], 0.0, op0=ALU.add, op1=ALU.max)

        NKTP = SK_T // 2  # ktile pairs per block

        def _exp_op(pT, psS, dve):
            if dve:
                nc.vector.tensor_scalar(
                    pT.bitcast(I8), psS, EXP8_A, EXP8_B,
                    op0=ALU.mult, op1=ALU.add)
            else:
                nc.scalar.activation(pT, psS, AF.Exp, bias=negc0,
                                     scale=0.125)

        def attn_kt_exp(j, qc, kt, pT_big, dve):
            """Scores (heads A||B, paired PE row groups, one shared psS tile
            so the pair is released atomically by ONE exp) + exp for ktile
            kt, written into plane kt%2 of pT_big."""
            q0 = qc * QC
            psS = psum.tile([P, 2 * QC], F32, tag="scores", bufs=2,
                            name="psS")
            for h in range(2):
                hp = h * DH
                nc.tensor.matmul(
                    psS[:, h * QC:(h + 1) * QC],
                    KT[hp:hp + DH, j, kt * P:(kt + 1) * P],
                    QT[hp:hp + DH, j, q0:q0 + QC], start=True, stop=True)
            _exp_op(pT_big[:, kt % 2], psS, dve)

        def attn_pair(j, qc, ktp, force_act=False, pt_tag="pT", pt_bufs=5):
            """One ktile pair: two scores+exp rounds into a shared
            [P, 2(kt), 2(h), QC] fp8 tile — the layout the DoubleRow U
            matmul consumes per head."""
            pT_big = work.tile([P, 2, 2, QC], FP8, tag=pt_tag, bufs=pt_bufs,
                               name=pt_tag)
            for i in range(2):
                kt = 2 * ktp + i
                dve = (not force_act) and (kt % 8) in DVE_RES
                attn_kt_exp(j, qc, kt, pT_big, dve)
            return pT_big

        def attn_u(j, ktp, pT_big, psU, h):
            """One DoubleRow fp8 matmul: psU[h] += V(kt).T P(kt) summed over
            the pair's 2 ktiles."""
            first, last = (ktp == 0), (ktp == NKTP - 1)
            nc.tensor.matmul(psU,
                             V_pad[:, 2 * ktp:2 * ktp + 2, 2 * j + h,
                                   0:DH + 1],
                             pT_big[:, :, h, :],
                             start=first, stop=last, perf_mode=DR)

        def u_pair(j, ktp, pT_big, psU):
            attn_u(j, ktp, pT_big, psU[0], 0)
            attn_u(j, ktp, pT_big, psU[1], 1)

        QCP = QC // P  # denom row [1, QC] reshapes to [P, QCP] for recip

        def attn_finish_copies(psU_A, psU_B):
            """Copy U out of PSUM fast — frees both accumulators for the
            next block.  Also DMA-gathers the two denominator rows into a
            [P, 2*QCP] collector so the reciprocal can run partition-parallel
            (a [1, QC] reciprocal serializes in one DVE lane).  Returns the
            SBUF copies and the collector."""
            ucs = []
            for psU in (psU_A, psU_B):
                uc = work.tile([DH + 1, QC], F32, tag="ucopy", bufs=6,
                               name="uc")
                nc.vector.tensor_copy(uc, psU)
                ucs.append(uc)
            den_t = work.tile([P, 2 * QCP], F32, tag="den", bufs=3,
                              name="den_t")
            for i, uc in enumerate(ucs):
                nc.sync.dma_start(den_t[:, i * QCP:(i + 1) * QCP],
                                  uc[DH:DH + 1, :])
            return ucs, den_t

        brc_sink = {}

        def normalize_thunks(j, qc, ucs, den_t, tail=False):
            """Per-head softmax normalize emitted later (as fillers inside
            the next block) so its latency hides under ACT-bound stretches.
            In tail mode the free-dim broadcast of the reciprocal row runs as
            a K=1 PE matmul into PSUM instead of the gpsimd partition
            broadcast — ~3us less latency on the final critical chain."""
            q0 = qc * QC
            rec_t = work.tile([P, 2 * QCP], F32, tag="rec", bufs=2,
                              name="rec_t")
            rrow2 = work.tile([1, 2 * QC], F32, tag="recip", bufs=3,
                              name="rrow2")
            brc2 = work.tile([DH, 2 * QC], F32, tag="brc", bufs=3,
                             name="brc2")

            def recip_bc():
                nc.vector.reciprocal_approx_fast(rec_t, den_t)
                for i in range(2):
                    nc.sync.dma_start(rrow2[:, i * QC:(i + 1) * QC],
                                      rec_t[:, i * QCP:(i + 1) * QCP])
                nc.gpsimd.partition_broadcast(brc2, rrow2)

            def one(uc, h0, i):
                def t():
                    nc.vector.tensor_mul(
                        OT[h0:h0 + DH, j, q0:q0 + QC], uc[0:DH, :],
                        brc2[:, i * QC:(i + 1) * QC])
                    brc_sink[(j, qc)] = brc2
                return t
            return [recip_bc, one(ucs[0], 0, 0), one(ucs[1], DH, 1)]

        def attn_span(j, qc, ktps, psU, fillers=(), precomputed=()):
            """Emit the exp groups of one attention block, sprinkling
            `fillers` (deferred work thunks) between groups so the in-order
            PE/DVE do them inside this ACT-bound stretch.  Returns this
            block's normalize thunks (to be run as fillers of the NEXT
            block)."""
            fillers = list(fillers)
            for pktp, ptile in precomputed:
                u_pair(j, pktp, ptile, psU)
            ngroups = 2 * len(ktps)
            spacing = max(1, ngroups // (len(fillers) + 1))
            gi = 0
            prev = None
            for ktp in ktps:
                # scores + exps for this pair first; the PREVIOUS pair's U
                # matmuls after, so the PE never sits in-queue behind an
                # in-flight exp (U(p) waits on exp(p); deferring it one pair
                # keeps the PE stream dependency-free)
                pTs = attn_pair(j, qc, ktp)
                if prev is not None:
                    u_pair(j, prev[0], prev[1], psU)
                prev = (ktp, pTs)
                for g in (gi + 1, gi + 2):
                    if fillers and g % spacing == 0:
                        fillers.pop(0)()
                gi += 2
            for f in fillers:
                f()
            u_pair(j, prev[0], prev[1], psU)
            if ktps[-1] == NKTP - 1:
                ucs, den_t = attn_finish_copies(psU[0], psU[1])
                tail = (j, qc) == (DT - 1, NQC - 1)
                return normalize_thunks(j, qc, ucs, den_t, tail=tail), ucs
            return [], None

        def new_psU():
            a = psum.tile([DH + 1, QC], F32, tag="psU", name="psU_A")
            b = psum.tile([DH + 1, QC], F32, tag="psU", name="psU_B")
            return (a, b)

        def outproj(qt):
            # bias matmul first: it reads OT1, whose re-write after the last
            # normalize acts as a scheduling gate for the whole chain (the
            # scheduler otherwise hoists these into mid-attention PE-idle
            # slots and stalls on under-modeled reciprocal latency)
            psO = psum.tile([P, D], F32, tag="proj", name="psO")
            nc.tensor.matmul(psO, OT1[:, qt * P:(qt + 1) * P],
                             b_row["bo"], start=True, stop=False)
            for j in range(DT):
                nc.tensor.matmul(psO, OT[:, j, qt * P:(qt + 1) * P],
                                 w_bf["wo"][:, j, :],
                                 start=False, stop=(j == DT - 1))
            o_sb = work.tile([P, D], F32, tag="osb", bufs=2, name="o_sb")
            if qt % 2 == 0 and qt >= SQ_T // NQC:
                # tail outprojs: half the relus on DVE so the two engines
                # drain the final chains in parallel
                nc.vector.tensor_scalar_max(o_sb, psO, 0.0)
            else:
                nc.scalar.activation(o_sb, psO, AF.Relu)
            nc.sync.dma_start(out[qt * P:(qt + 1) * P, :], o_sb)

        def gate_outproj(blk):
            """No-op rewrite of OT1 (max(1, recip<1) == 1) that depends on
            block `blk`'s normalize chain — gates the outproj chains (which
            start with an OT1-reading bias matmul) behind it, preventing the
            scheduler from hoisting them into mid-attention stalls."""
            brc = brc_sink[blk]
            nc.vector.tensor_scalar(OT1, OT1, brc[0:1, 0:1], None,
                                    op0=ALU.max)

        # ---- chunk loop: x load + V proj + K proj(pair 0) + attn(0, 0) ----
        psU0 = new_psU()
        N_STORE = 8
        store01 = []
        store01_kts = [0]
        store01_cur = [None]
        pendq = []   # queue of deferred normalize-thunk lists (2-block lag)
        # chunk structure: tiny leading chunks get the first exps going
        # ~10us earlier (stile-major x layout makes any stile range one
        # linear DMA); 4-stile chunks once the pipeline is primed
        CHUNKS = [[0], [1], [2, 3]] + [
            list(range(4 * k, 4 * k + 4)) for k in range(1, NCH)]
        cur00 = [None]
        for ci, ch in enumerate(CHUNKS):
            if ci >= 3:
                nc.sync.dma_start(xT[:, ch[0]:ch[-1] + 1],
                                  xT_src[:, ch[0]:ch[-1] + 1])
                nc.sync.dma_start(x8[:, ch[0]:ch[-1] + 1],
                                  x8_src[:, ch[0]:ch[-1] + 1])
            kproj(0, ch[0], len(ch))
            # QK + exp first: ACT can start before V exists (only U needs V)
            completed = []
            for kt in ch:
                if kt % 2 == 0:
                    cur00[0] = work.tile([P, 2, 2, QC], FP8, tag="pT",
                                         bufs=5, name="pT")
                dve00 = kt >= SK_T // 2 and (kt % 8) in DVE_RES
                attn_kt_exp(0, 0, kt, cur00[0], dve00)
                if kt % 2 == 1:
                    completed.append((kt // 2, cur00[0]))
            for st in ch:
                vproj(st)
            for ktp, pp in completed:
                u_pair(0, ktp, pp, psU0)
            # pre-compute exps of block (0,1) into held pT tiles: fills the
            # otherwise-idle ACT during the PE-bound chunk phase (the U
            # matmuls run later, so no PSUM cost).  Stored ktile idx <=
            # ch[-1], so its K tiles already exist; a pT01 tile is recorded
            # once both of its ktile planes are in.
            while (NQC > 1 and ci >= 3 and store01_kts[0] < N_STORE
                   and store01_kts[0] <= ch[-1]
                   and store01_kts[0] < (ci - 2) * N_STORE // (len(CHUNKS)
                                                               - 3) + 1):
                kt_s = store01_kts[0]
                if kt_s % 2 == 0:
                    store01_cur[0] = work.tile([P, 2, 2, QC], FP8,
                                               tag="pT01",
                                               bufs=N_STORE // 2,
                                               name="pT01")
                attn_kt_exp(0, 1, kt_s, store01_cur[0], False)
                if kt_s % 2 == 1:
                    store01.append((kt_s // 2, store01_cur[0]))
                store01_kts[0] += 1
            if ch[-1] == SK_T - 1:
                ucs0, den_t0 = attn_finish_copies(psU0[0], psU0[1])
                thunks = normalize_thunks(0, 0, ucs0, den_t0)
        pendq.append(thunks)

        # ---- remaining attention; fillers inside each ACT-bound block are:
        # the previous block's normalize chain + the next block's
        # projections (+ the qc0 half of the output projection during the
        # last block) ----
        blocks = [(0, qc) for qc in range(1, NQC)]
        blocks += [(j, qc) for j in range(1, DT) for qc in range(NQC)]
        owed = {blk: [] for blk in blocks}
        for (j, qc) in blocks:
            if (j, qc) != (0, 1):
                owed[(j, qc)].append(lambda j=j, qc=qc: qproj(j, qc))
            if qc == 0 and j >= 1:
                for n in range(NCH):
                    owed[(j, qc)].append(lambda j=j, n=n: kproj(j, 4 * n, 4))
        for f in owed[blocks[0]]:
            f()
        for bi, (j, qc) in enumerate(blocks):
            # projection fillers first; normalize chains run with a 2-block
            # lag so their slow DVE reciprocals never sit near a block
            # boundary (where they would delay the relus feeding the next
            # pair's attention)
            fillers = []
            if bi + 1 < len(blocks):
                fillers += owed[blocks[bi + 1]]
            last = bi == len(blocks) - 1
            if last:
                # flush remaining normalize chains, then gate + emit the qc0
                # half of the output projection so it runs inside this block
                while pendq:
                    fillers += pendq.pop(0)
                if NQC > 1:
                    fillers += [lambda: gate_outproj((DT - 1, 0))]
                    fillers += [lambda qt=qt: outproj(qt)
                                for qt in range(SQ_T // NQC)]
            elif len(pendq) >= 2:
                fillers += pendq.pop(0)
                if bi == len(blocks) - 2 and pendq:
                    fillers += pendq.pop(0)
            psU = new_psU()
            if (j, qc) == (0, 1) and store01:
                thunks, ucs = attn_span(
                    j, qc, list(range(len(store01), NKTP)), psU,
                    fillers, precomputed=store01)
            else:
                thunks, ucs = attn_span(j, qc, list(range(NKTP)), psU,
                                        fillers)
            pendq.append(thunks)
            last_ucs = ucs

        # ---- tail: last block's normalize + remaining output rows ----
        # Two of the final outproj chains are gated only on the last block's
        # PSUM copies (their bias + pairs-0..2 matmuls need nothing newer),
        # so the PE does useful work during the slow reciprocal chain and
        # stays HAM-warm; their pair-3 matmul still waits on the real OT
        # write.  Gate writes go on DVE BEFORE the normalize thunks so they
        # are not queued behind the reciprocals.
        qt_lo = SQ_T // NQC if NQC > 1 else 0
        early = []
        open_psO = []
        if NQC > 1 and last_ucs is not None:
            early = [qt_lo, qt_lo + 1]
            for qt, uc in zip(early, last_ucs):
                nc.vector.tensor_scalar(
                    OT1[:, qt * P:(qt + 1) * P],
                    OT1[:, qt * P:(qt + 1) * P],
                    uc[DH:DH + 1, 0:1], None, op0=ALU.min)
            # partial chains (bias + pairs 0..2): no pair-3 matmul yet, so
            # the in-order PE runs all 8 matmuls during the reciprocals
            # instead of stalling at the first chain's pair-3 wait
            for qt in early:
                psO = psum.tile([P, D], F32, tag="proj", name="psO")
                nc.tensor.matmul(psO, OT1[:, qt * P:(qt + 1) * P],
                                 b_row["bo"], start=True, stop=False)
                for j in range(DT - 1):
                    nc.tensor.matmul(psO, OT[:, j, qt * P:(qt + 1) * P],
                                     w_bf["wo"][:, j, :],
                                     start=False, stop=False)
                open_psO.append((qt, psO))
        while pendq:
            for f in pendq.pop(0):
                f()
        for qt, psO in open_psO:
            nc.tensor.matmul(psO, OT[:, DT - 1, qt * P:(qt + 1) * P],
                             w_bf["wo"][:, DT - 1, :],
                             start=False, stop=True)
            o_sb = work.tile([P, D], F32, tag="osb", bufs=2, name="o_sb")
            if qt % 2 == 0:
                nc.vector.tensor_scalar_max(o_sb, psO, 0.0)
            else:
                nc.scalar.activation(o_sb, psO, AF.Relu)
            nc.sync.dma_start(out[qt * P:(qt + 1) * P, :], o_sb)
        gate_outproj(blocks[-1])
        for qt in range(qt_lo, SQ_T):
            if qt not in early:
                outproj(qt)


_NC_CACHE = {}


def _get_nc(sk=S, sq=SQ_FULL, skip_vbias=False):
    key = (sk, sq, skip_vbias)
    if key not in _NC_CACHE:
        _NC_CACHE[key] = build_mha(sk, sq, skip_vbias)
    return _NC_CACHE[key]


def _tile_rows(a):
    """[D, n] -> SBUF layout [P, DT*n]: partition p gets rows p, 128+p, ..."""
    Dd, n = a.shape
    t = Dd // P
    return np.ascontiguousarray(
        a.reshape(t, P, n).transpose(1, 0, 2).reshape(P, t * n))


def _tile_rows_j(a):
    """[D, D] -> SBUF layout [P, DT(j)*DT(t)*P]: output-tile-major so each
    128-col output slice (one head pair's weights) is one contiguous DMA."""
    Dd, n = a.shape
    t, nj = Dd // P, n // P
    return np.ascontiguousarray(
        a.reshape(t, P, nj, P).transpose(1, 2, 0, 3).reshape(P, -1))


def _tile_chunks(a, chp):
    """[D, sk] -> chunk-major SBUF layout [P, NCH*DT*chp]: per partition,
    sequence chunks outermost so each chunk is one contiguous linear DMA."""
    Dd, sk = a.shape
    t, nch = Dd // P, sk // chp
    return np.ascontiguousarray(
        a.reshape(t, P, nch, chp).transpose(1, 2, 0, 3).reshape(P, -1))


def prep_inputs(x, Wq, bq, Wk, bk, Wv, bv, Wo, bo):
    """Host-side sharding/layout prep: bf16 casts, feature-major transpose,
    SBUF pre-tiling.  Returns the 8 per-core input maps."""
    bf = ml_dtypes.bfloat16
    f8 = ml_dtypes.float8_e4m3
    x = np.asarray(x, dtype=np.float32)
    shared = {
        "wq": _tile_rows_j(np.asarray(Wq, np.float32).astype(bf)),
        "wk": _tile_rows_j(np.asarray(Wk, np.float32).astype(bf)),
        "wv": _tile_rows(np.asarray(Wv, np.float32).astype(f8)),
        "wo": _tile_rows(np.asarray(Wo, np.float32).astype(bf)),
        "bq": np.ascontiguousarray(
            np.asarray(bq, np.float32).reshape(DT, P).T),
        "bk": np.ascontiguousarray(
            np.asarray(bk, np.float32).reshape(DT, P).T),
        "bv": np.asarray(bv, np.float32).astype(f8).reshape(1, D),
        "bo": np.asarray(bo, np.float32).astype(bf).reshape(1, D),
    }
    xT_b = [x[b].T.astype(bf) for b in range(B)]
    in_maps = []
    for c in range(NCORES):
        b, qo = divmod(c, QSPLIT)
        m = dict(shared)
        m["xT_bf"] = _tile_chunks(xT_b[b], P)
        m["xT_f8"] = _tile_chunks(xT_b[b].astype(f8), P)
        m["xqT_bf"] = _tile_rows(
            xT_b[b][:, qo * SQ_FULL:(qo + 1) * SQ_FULL])
        in_maps.append(m)
    return in_maps


def kernel(x, Wq, bq, Wk, bk, Wv, bv, Wo, bo, **run_kwargs):
    """Full-input entry point: shards across 8 NeuronCores, returns full out."""
    in_maps = prep_inputs(x, Wq, bq, Wk, bk, Wv, bv, Wo, bo)
    nc = _get_nc(skip_vbias=bool(np.all(np.asarray(bv) == 0)))
    res = bass_utils.run_bass_kernel_spmd(
        nc, in_maps, core_ids=list(range(NCORES)), **run_kwargs)
    full = np.empty((B, S, D), np.float32)
    for c in range(NCORES):
        b, qo = divmod(c, QSPLIT)
        full[b, qo * SQ_FULL:(qo + 1) * SQ_FULL] = res.results[c]["out"]
    if run_kwargs:
        return full, res
    return full



# revision 52
# speedup vs baseline: 1.1936x; 1.0085x over previous
"""Trainium2 Bass kernel for nn_MultiHeadAttention (B=2, S=4096, D=512, H=8).

Computes: q/k/v = relu(x@W+b) per head, softmax(q k^T / sqrt(64)) v,
out = relu(concat_heads @ Wo + bo).

Sharding: 8 cores = 2 (batch) x 4 (query-slice).  Each core computes full
K/V projections for its batch (redundant across the 4 q-slice cores) and
attention + output projection for its 1024-row query slice.  No collectives;
the host concatenates the 8 output slices.

Host-side prep (part of the sharding/layout step, not device compute):
x is cast to bf16 and transposed to feature-major x^T per batch, and the
weight matrices are cast to bf16 — the tensor engine contracts along the
partition dim, so all device matmuls consume feature-major operands.

Per-core kernel (all matmuls bf16 with fp32 PSUM accumulation):
  - K^T, Q^T computed feature-major: lhsT=W tile, rhs=x^T.  Bias+relu fused
    on DVE (bias is per-partition in this layout).
  - V computed in natural [s, d] layout (lhsT = x^T tile, rhs = Wv); bias via
    a K=1 ones-row matmul; relu on DVE; stored per head with a ones column
    appended (V_pad) so the attention U matmul also produces the softmax
    denominator row for free.
  - scores^T = K^T_h.T @ Q^T_h per (head, ktile): K=64 contraction; heads are
    processed in pairs at base partitions 0/64 so the two matmuls run
    concurrently in different PE row-groups.
  - exp on ACT (scale=1/8 fused), no max-subtraction (relu'd q/k make scores
    bounded: measured range [0, 6.6]).  ACT exp is the kernel's throughput
    floor (~1 elem/lane/cycle): exp ops span 2 ktiles x 2 heads (4 PSUM
    banks) to amortize the per-op overhead, the first attention block is
    interleaved with the K/V projection chunks, and the remaining
    projections are emitted between attention blocks so the PE does them
    inside ACT-bound stretches.
  - U^T[65, q] = V_pad_h.T @ P^T accumulated over ktiles in PSUM; row 64 is
    the denominator.  U^T is copied to SBUF immediately (releases the PSUM
    accumulator for the next block), then normalized off the critical path:
    DVE reciprocal + gpsimd partition broadcast + DVE multiply into
    feature-major O^T.
  - out = relu(O^T.T @ Wo + bo) via lhsT=O^T tiles, rhs=Wo; bias via ones-row
    matmul; relu on ACT; DMA to HBM.
"""

import numpy as np
import ml_dtypes

import concourse.bass as bass
import concourse.mybir as mybir
import concourse.tile as tile
from concourse import bacc
from concourse import bass_utils

F32 = mybir.dt.float32
BF16 = mybir.dt.bfloat16
FP8 = mybir.dt.float8e4
I8 = mybir.dt.int8
AF = mybir.ActivationFunctionType
ALU = mybir.AluOpType
DR = mybir.MatmulPerfMode.DoubleRow

P = 128
D = 512
H = 8
DH = 64
DT = D // P  # 4 (also = number of head pairs)
B = 2
S = 4096
NCORES = 8
QSPLIT = 4
SQ_FULL = S // QSPLIT  # 1024 query rows per core
QC = 512               # q-chunk (matmul free dim / PSUM bank width)

# ---- exp offload + fp8 attention weights.
# pT = exp(s/8 - 2) stored in fp8 e4m3 (the -2 shift keeps values in
# [e^-2, ~e^4.6], inside e4m3 range; softmax is shift-invariant since the
# denominator uses the same shifted exps).  Two producers:
#   ACT: exact exp via activation(scale=1/8, bias=-2) with fp8 output.
#   DVE (a fraction of groups, to unload the ACT bottleneck): Schraudolph bit
#     trick — e4m3(exp(s/8-2)) bit pattern ~= int8 round of A8*s + B8.
#     Scores are >= 0 (relu'd q,k) and bounded (~53 pre-scale), so the affine
#     stays in [32, ~110] (NaN at 120+), and the ~4% sawtooth error cancels
#     in the softmax ratio (validated <6e-3 end-to-end incl. fp8 V).
C0_SHIFT = 2.0
EXP8_A = 8.0 * 0.125 * 1.4426950408889634
EXP8_B = 56.0 - 8.0 * C0_SHIFT * 1.4426950408889634 - 0.35
# which exp ktiles of each steady attention block go to DVE (kt % 8)
DVE_RES = (1, 3, 6)


def build_mha(sk=S, sq=SQ_FULL, skip_vbias=False):
    """Build the SPMD Bass program (identical on all cores).

    All inputs arrive pre-tiled by the host into exact SBUF layout
    ([128 partitions, contiguous free bytes]) so every load is a max-packet
    linear DMA."""
    nc = bacc.Bacc("TRN2", target_bir_lowering=False, debug=False,
                   num_devices=NCORES)

    xT_d = nc.dram_tensor("xT_bf", (P, DT * sk), BF16,
                          kind="ExternalInput").ap()  # stile-major, see prep
    x8_d = nc.dram_tensor("xT_f8", (P, DT * sk), FP8,
                          kind="ExternalInput").ap()  # stile-major fp8 copy
    xqT_d = nc.dram_tensor("xqT_bf", (P, DT * sq), BF16,
                           kind="ExternalInput").ap()
    w_dram = {}
    for n in ("wq", "wk", "wo"):
        w_dram[n] = nc.dram_tensor(n, (P, DT * D), BF16,
                                   kind="ExternalInput").ap()
    w_dram["wv"] = nc.dram_tensor("wv", (P, DT * D), FP8,
                                  kind="ExternalInput").ap()
    b_dram = {
        "bq": nc.dram_tensor("bq", (P, DT), F32, kind="ExternalInput").ap(),
        "bk": nc.dram_tensor("bk", (P, DT), F32, kind="ExternalInput").ap(),
        "bv": nc.dram_tensor("bv", (1, D), FP8, kind="ExternalInput").ap(),
        "bo": nc.dram_tensor("bo", (1, D), BF16, kind="ExternalInput").ap(),
    }
    out = nc.dram_tensor("out", (sq, D), F32, kind="ExternalOutput").ap()

    with tile.TileContext(nc) as tc:
        _build_tile(tc, xT_d, x8_d, xqT_d, w_dram, b_dram, out, sk, sq,
                    skip_vbias)

    nc.compile()
    return nc


def _build_tile(tc, xT_d, x8_d, xqT_d, w_dram, b_dram, out, sk, sq,
                skip_vbias=False):
    nc = tc.nc
    SK_T = sk // P            # ktiles of the key/value sequence
    SQ_T = sq // P
    NQC = sq // QC            # q chunks per core
    CH = min(4, SK_T)         # stiles per projection chunk
    NCH = SK_T // CH
    KG = 1                    # ktiles per exp group

    with (
        tc.tile_pool(name="singles", bufs=1) as singles,
        tc.tile_pool(name="work", bufs=3) as work,
        tc.tile_pool(name="psum", bufs=2, space="PSUM") as psum,
    ):
        # ---- startup: DMAs issued in exactly the order the critical path
        # consumes them (queue executes in issue order).  wq/wk arrive in
        # j-major layout so the pair-0 slices are single contiguous DMAs;
        # x^T arrives stile-major so the first ktile is a small early DMA.
        w_bf = {}
        b_col = {}
        wq_src = w_dram["wq"].rearrange("p (j t c) -> p j t c", j=DT, t=DT)
        wk_src = w_dram["wk"].rearrange("p (j t c) -> p j t c", j=DT, t=DT)
        w_bf["wq"] = singles.tile([P, DT, DT, P], BF16, name="wq_bf")
        w_bf["wk"] = singles.tile([P, DT, DT, P], BF16, name="wk_bf")
        nc.sync.dma_start(w_bf["wq"][:, 0], wq_src[:, 0])
        b_col["bq"] = singles.tile([P, DT], F32, name="bq_col")
        nc.sync.dma_start(b_col["bq"], b_dram["bq"])
        xTq = singles.tile([P, DT, sq], BF16)
        xTq_src = xqT_d.rearrange("p (t s) -> p t s", t=DT)
        nc.sync.dma_start(xTq[:, :, 0:QC], xTq_src[:, :, 0:QC])
        # kproj-path inputs ride the ACT hardware DMA queue (idle until the
        # first exp) so they transfer in parallel with the qproj path above
        nc.scalar.dma_start(w_bf["wk"][:, 0], wk_src[:, 0])
        b_col["bk"] = singles.tile([P, DT], F32, name="bk_col")
        nc.scalar.dma_start(b_col["bk"], b_dram["bk"])
        xT = singles.tile([P, SK_T, DT, P], BF16)
        xT_src = xT_d.rearrange("p (s t c) -> p s t c", s=SK_T, t=DT)
        x8 = singles.tile([P, SK_T, DT, P], FP8)
        x8_src = x8_d.rearrange("p (s t c) -> p s t c", s=SK_T, t=DT)
        nc.scalar.dma_start(xT[:, 0:1], xT_src[:, 0:1])
        nc.scalar.dma_start(xT[:, 1:2], xT_src[:, 1:2])
        nc.scalar.dma_start(xT[:, 2:4], xT_src[:, 2:4])
        nc.scalar.dma_start(x8[:, 0:2], x8_src[:, 0:2])
        nc.scalar.dma_start(x8[:, 2:4], x8_src[:, 2:4])

        # ---- persistent SBUF tensors (memsets early: the warmup matmuls
        # below need xT1 before the input DMAs land) ----
        xT1 = singles.tile([1, sk], FP8)
        nc.vector.memset(xT1, 1.0)
        KT = singles.tile([P, DT, sk], BF16)
        # per-head width padded 65->66 so the DoubleRow weight-pair stride
        # (H*66 = 528 B) meets the 16B-alignment ISA restriction
        V_pad = singles.tile([P, SK_T, H, DH + 2], FP8)
        nc.vector.memset(V_pad[:, :, :, DH:DH + 1], 1.0)
        OT = singles.tile([P, DT, sq], BF16)
        OT1 = singles.tile([1, sq], BF16)
        nc.vector.memset(OT1, 1.0)
        negc0 = singles.tile([P, 1], F32, name="neg_c0")
        nc.vector.memset(negc0, -C0_SHIFT)
        ones64 = singles.tile([1, DH], BF16, name="ones64")
        nc.vector.memset(ones64, 1.0)
        # dummy K=1 matmuls during the input-DMA wait: sustained PE activity
        # releases the HAM clock gate, so the startup projection chain and
        # early chunks run at full clock instead of 4/8
        for _ in range(8):
            psW = psum.tile([P, 2 * QC], F32, tag="scores", name="psS")
            nc.tensor.matmul(psW[:, 0:QC], xT1[0:1, 0:P], xT1[0:1, 0:QC],
                             start=True, stop=True)

        QT = singles.tile([P, DT, sq], BF16)

        def qproj(j, nq):
            psQ = psum.tile([P, QC], F32, tag="proj", name="psQ")
            for kt in range(DT):
                nc.tensor.matmul(
                    psQ, w_bf["wq"][:, j, kt, :],
                    xTq[:, kt, nq * QC:(nq + 1) * QC],
                    start=(kt == 0), stop=(kt == DT - 1))
            nc.vector.tensor_scalar(
                QT[:, j, nq * QC:(nq + 1) * QC], psQ,
                b_col["bq"][:, j:j + 1], 0.0, op0=ALU.add, op1=ALU.max)

        qproj(0, 0)

        # ---- bulk input DMAs, still roughly in consumption order ----
        b_row = {}
        if NQC > 1:
            nc.sync.dma_start(xTq[:, :, QC:sq], xTq_src[:, :, QC:sq])
            qproj(0, 1)
        wb = singles.tile([P, DT, D], FP8, name="wv_bf")
        nc.sync.dma_start(wb, w_dram["wv"].rearrange("p (t n) -> p t n",
                                                     t=DT))
        w_bf["wv"] = wb
        br = singles.tile([1, D], FP8, name="bv_row")
        nc.sync.dma_start(br, b_dram["bv"])
        b_row["bv"] = br
        for jj in range(1, DT):
            nc.sync.dma_start(w_bf["wk"][:, jj], wk_src[:, jj])
            nc.sync.dma_start(w_bf["wq"][:, jj], wq_src[:, jj])
        wb = singles.tile([P, DT, D], BF16, name="wo_bf")
        nc.sync.dma_start(wb, w_dram["wo"].rearrange("p (t n) -> p t n",
                                                     t=DT))
        w_bf["wo"] = wb
        br = singles.tile([1, D], BF16, name="bo_row")
        nc.sync.dma_start(br, b_dram["bo"])
        b_row["bo"] = br
        CHP = CH * P


        # PSUM tags: "proj" 2x1 banks, "scores" 1x4 banks, "psU" 2x1 = 8
        def vproj(st):
            psV = psum.tile([P, D], F32, tag="proj", name="psV")
            for tp in range(DT // 2):
                nc.tensor.matmul(
                    psV, x8[:, st, 2 * tp:2 * tp + 2, :],
                    w_bf["wv"][:, 2 * tp:2 * tp + 2, :],
                    start=(tp == 0),
                    stop=(skip_vbias and tp == DT // 2 - 1), perf_mode=DR)
            if not skip_vbias:
                nc.tensor.matmul(psV, xT1[:, st * P:(st + 1) * P],
                                 b_row["bv"], start=False, stop=True)
            nc.vector.tensor_scalar_max(
                V_pad[:, st, :, 0:DH],
                psV.rearrange("p (h d) -> p h d", h=H), 0.0)

        def kproj(j, st0, nst):
            psK = psum.tile([P, CHP], F32, tag="proj", name="psK")
            for kt in range(DT):
                nc.tensor.matmul(
                    psK[:, 0:nst * P], w_bf["wk"][:, j, kt, :],
                    xT[:, st0:st0 + nst, kt, :],
                    start=(kt == 0), stop=(kt == DT - 1))
            nc.vector.tensor_scalar(
                KT[:, j, st0 * P:(st0 + nst) * P], psK[:, 0:nst * P],
                b_col["bk"][:, j:j + 1], 0.0, op0=ALU.add, op1=ALU.max)

        NKTP = SK_T // 2  # ktile pairs per block

        def _exp_op(pT, psS, dve):
            if dve:
                nc.vector.tensor_scalar(
                    pT.bitcast(I8), psS, EXP8_A, EXP8_B,
                    op0=ALU.mult, op1=ALU.add)
            else:
                nc.scalar.activation(pT, psS, AF.Exp, bias=negc0,
                                     scale=0.125)

        def attn_kt_exp(j, qc, kt, pT_big, dve):
            """Scores (heads A||B, paired PE row groups, one shared psS tile
            so the pair is released atomically by ONE exp) + exp for ktile
            kt, written into plane kt%2 of pT_big."""
            q0 = qc * QC
            psS = psum.tile([P, 2 * QC], F32, tag="scores", bufs=2,
                            name="psS")
            for h in range(2):
                hp = h * DH
                nc.tensor.matmul(
                    psS[:, h * QC:(h + 1) * QC],
                    KT[hp:hp + DH, j, kt * P:(kt + 1) * P],
                    QT[hp:hp + DH, j, q0:q0 + QC], start=True, stop=True)
            _exp_op(pT_big[:, kt % 2], psS, dve)

        def attn_pair(j, qc, ktp, force_act=False, pt_tag="pT", pt_bufs=5):
            """One ktile pair: two scores+exp rounds into a shared
            [P, 2(kt), 2(h), QC] fp8 tile — the layout the DoubleRow U
            matmul consumes per head."""
            pT_big = work.tile([P, 2, 2, QC], FP8, tag=pt_tag, bufs=pt_bufs,
                               name=pt_tag)
            for i in range(2):
                kt = 2 * ktp + i
                dve = (not force_act) and (kt % 8) in DVE_RES
                attn_kt_exp(j, qc, kt, pT_big, dve)
            return pT_big

        def attn_u(j, ktp, pT_big, psU, h):
            """One DoubleRow fp8 matmul: psU[h] += V(kt).T P(kt) summed over
            the pair's 2 ktiles."""
            first, last = (ktp == 0), (ktp == NKTP - 1)
            nc.tensor.matmul(psU,
                             V_pad[:, 2 * ktp:2 * ktp + 2, 2 * j + h,
                                   0:DH + 1],
                             pT_big[:, :, h, :],
                             start=first, stop=last, perf_mode=DR)

        def u_pair(j, ktp, pT_big, psU):
            attn_u(j, ktp, pT_big, psU[0], 0)
            attn_u(j, ktp, pT_big, psU[1], 1)

        QCP = QC // P  # denom row [1, QC] reshapes to [P, QCP] for recip

        def attn_finish_copies(psU_A, psU_B):
            """Copy U out of PSUM fast — frees both accumulators for the
            next block.  Also DMA-gathers the two denominator rows into a
            [P, 2*QCP] collector so the reciprocal can run partition-parallel
            (a [1, QC] reciprocal serializes in one DVE lane).  Returns the
            SBUF copies and the collector."""
            ucs = []
            for psU in (psU_A, psU_B):
                uc = work.tile([DH + 1, QC], F32, tag="ucopy", bufs=6,
                               name="uc")
                nc.vector.tensor_copy(uc, psU)
                ucs.append(uc)
            den_t = work.tile([P, 2 * QCP], F32, tag="den", bufs=3,
                              name="den_t")
            for i, uc in enumerate(ucs):
                nc.sync.dma_start(den_t[:, i * QCP:(i + 1) * QCP],
                                  uc[DH:DH + 1, :])
            return ucs, den_t

        brc_sink = {}

        def normalize_thunks(j, qc, ucs, den_t, tail=False):
            """Per-head softmax normalize emitted later (as fillers inside
            the next block) so its latency hides under ACT-bound stretches.
            In tail mode the free-dim broadcast of the reciprocal row runs as
            a bf16 K=1 PE matmul into a (then-free) psU PSUM bank instead of
            the gpsimd partition broadcast — ~2.5us less latency on the
            final critical chain (gpsimd queue + op + drain)."""
            q0 = qc * QC
            rec_t = work.tile([P, 2 * QCP], F32, tag="rec", bufs=2,
                              name="rec_t")
            if tail:
                rec_b = work.tile([P, 2 * QCP], BF16, tag="recb", bufs=1,
                                  name="rec_b")
                rrow2b = work.tile([1, 2 * QC], BF16, tag="recipb", bufs=1,
                                   name="rrow2b")
            else:
                rrow2 = work.tile([1, 2 * QC], F32, tag="recip", bufs=3,
                                  name="rrow2")
                brc2 = work.tile([DH, 2 * QC], F32, tag="brc", bufs=3,
                                 name="brc2")

            def recip_bc():
                nc.vector.reciprocal_approx_fast(rec_t, den_t)
                if tail:
                    nc.vector.tensor_copy(rec_b, rec_t)
                    for i in range(2):
                        nc.sync.dma_start(rrow2b[:, i * QC:(i + 1) * QC],
                                          rec_b[:, i * QCP:(i + 1) * QCP])
                    brc_sink[(j, qc)] = rec_t
                else:
                    for i in range(2):
                        nc.sync.dma_start(rrow2[:, i * QC:(i + 1) * QC],
                                          rec_t[:, i * QCP:(i + 1) * QCP])
                    nc.gpsimd.partition_broadcast(brc2, rrow2)

            def one(uc, h0, i):
                def t():
                    if tail:
                        psB = psum.tile([DH + 1, QC], F32, tag="psU",
                                        name="psU_A")
                        nc.tensor.matmul(psB[0:DH, :], ones64,
                                         rrow2b[:, i * QC:(i + 1) * QC],
                                         start=True, stop=True)
                        src_b = psB[0:DH, :]
                    else:
                        src_b = brc2[:, i * QC:(i + 1) * QC]
                        brc_sink[(j, qc)] = brc2
                    nc.vector.tensor_mul(
                        OT[h0:h0 + DH, j, q0:q0 + QC], uc[0:DH, :], src_b)
                return t
            return [recip_bc, one(ucs[0], 0, 0), one(ucs[1], DH, 1)]

        def attn_span(j, qc, ktps, psU, fillers=(), precomputed=()):
            """Emit the exp groups of one attention block, sprinkling
            `fillers` (deferred work thunks) between groups so the in-order
            PE/DVE do them inside this ACT-bound stretch.  Returns this
            block's normalize thunks (to be run as fillers of the NEXT
            block)."""
            fillers = list(fillers)
            for pktp, ptile in precomputed:
                u_pair(j, pktp, ptile, psU)
            ngroups = 2 * len(ktps)
            spacing = max(1, ngroups // (len(fillers) + 1))
            gi = 0
            prev = None
            for ktp in ktps:
                # scores + exps for this pair first; the PREVIOUS pair's U
                # matmuls after, so the PE never sits in-queue behind an
                # in-flight exp (U(p) waits on exp(p); deferring it one pair
                # keeps the PE stream dependency-free)
                pTs = attn_pair(j, qc, ktp)
                if prev is not None:
                    u_pair(j, prev[0], prev[1], psU)
                prev = (ktp, pTs)
                for g in (gi + 1, gi + 2):
                    if fillers and g % spacing == 0:
                        fillers.pop(0)()
                gi += 2
            for f in fillers:
                f()
            u_pair(j, prev[0], prev[1], psU)
            if ktps[-1] == NKTP - 1:
                ucs, den_t = attn_finish_copies(psU[0], psU[1])
                tail = (j, qc) == (DT - 1, NQC - 1)
                return normalize_thunks(j, qc, ucs, den_t, tail=tail), ucs
            return [], None

        def new_psU():
            a = psum.tile([DH + 1, QC], F32, tag="psU", name="psU_A")
            b = psum.tile([DH + 1, QC], F32, tag="psU", name="psU_B")
            return (a, b)

        def outproj(qt):
            # bias matmul first: it reads OT1, whose re-write after the last
            # normalize acts as a scheduling gate for the whole chain (the
            # scheduler otherwise hoists these into mid-attention PE-idle
            # slots and stalls on under-modeled reciprocal latency)
            psO = psum.tile([P, D], F32, tag="proj", name="psO")
            nc.tensor.matmul(psO, OT1[:, qt * P:(qt + 1) * P],
                             b_row["bo"], start=True, stop=False)
            for j in range(DT):
                nc.tensor.matmul(psO, OT[:, j, qt * P:(qt + 1) * P],
                                 w_bf["wo"][:, j, :],
                                 start=False, stop=(j == DT - 1))
            o_sb = work.tile([P, D], F32, tag="osb", bufs=2, name="o_sb")
            if qt % 2 == 0 and qt >= SQ_T // NQC:
                # tail outprojs: half the relus on DVE so the two engines
                # drain the final chains in parallel
                nc.vector.tensor_scalar_max(o_sb, psO, 0.0)
            else:
                nc.scalar.activation(o_sb, psO, AF.Relu)
            nc.sync.dma_start(out[qt * P:(qt + 1) * P, :], o_sb)

        def gate_outproj(blk):
            """No-op rewrite of OT1 (max(1, recip<1) == 1) that depends on
            block `blk`'s normalize chain — gates the outproj chains (which
            start with an OT1-reading bias matmul) behind it, preventing the
            scheduler from hoisting them into mid-attention stalls."""
            brc = brc_sink[blk]
            nc.vector.tensor_scalar(OT1, OT1, brc[0:1, 0:1], None,
                                    op0=ALU.max)

        # ---- chunk loop: x load + V proj + K proj(pair 0) + attn(0, 0) ----
        psU0 = new_psU()
        N_STORE = 8
        store01 = []
        store01_kts = [0]
        store01_cur = [None]
        pendq = []   # queue of deferred normalize-thunk lists (2-block lag)
        # chunk structure: tiny leading chunks get the first exps going
        # ~10us earlier (stile-major x layout makes any stile range one
        # linear DMA); 4-stile chunks once the pipeline is primed
        CHUNKS = [[0], [1], [2, 3]] + [
            list(range(4 * k, 4 * k + 4)) for k in range(1, NCH)]
        cur00 = [None]
        for ci, ch in enumerate(CHUNKS):
            if ci >= 3:
                nc.sync.dma_start(xT[:, ch[0]:ch[-1] + 1],
                                  xT_src[:, ch[0]:ch[-1] + 1])
                nc.sync.dma_start(x8[:, ch[0]:ch[-1] + 1],
                                  x8_src[:, ch[0]:ch[-1] + 1])
            kproj(0, ch[0], len(ch))
            # QK + exp first: ACT can start before V exists (only U needs V)
            completed = []
            for kt in ch:
                if kt % 2 == 0:
                    cur00[0] = work.tile([P, 2, 2, QC], FP8, tag="pT",
                                         bufs=5, name="pT")
                dve00 = kt >= SK_T // 2 and (kt % 8) in DVE_RES
                attn_kt_exp(0, 0, kt, cur00[0], dve00)
                if kt % 2 == 1:
                    completed.append((kt // 2, cur00[0]))
            for st in ch:
                vproj(st)
            for ktp, pp in completed:
                u_pair(0, ktp, pp, psU0)
            # pre-compute exps of block (0,1) into held pT tiles: fills the
            # otherwise-idle ACT during the PE-bound chunk phase (the U
            # matmuls run later, so no PSUM cost).  Stored ktile idx <=
            # ch[-1], so its K tiles already exist; a pT01 tile is recorded
            # once both of its ktile planes are in.
            while (NQC > 1 and ci >= 3 and store01_kts[0] < N_STORE
                   and store01_kts[0] <= ch[-1]
                   and store01_kts[0] < (ci - 2) * N_STORE // (len(CHUNKS)
                                                               - 3) + 1):
                kt_s = store01_kts[0]
                if kt_s % 2 == 0:
                    store01_cur[0] = work.tile([P, 2, 2, QC], FP8,
                                               tag="pT01",
                                               bufs=N_STORE // 2,
                                               name="pT01")
                attn_kt_exp(0, 1, kt_s, store01_cur[0], False)
                if kt_s % 2 == 1:
                    store01.append((kt_s // 2, store01_cur[0]))
                store01_kts[0] += 1
            if ch[-1] == SK_T - 1:
                ucs0, den_t0 = attn_finish_copies(psU0[0], psU0[1])
                thunks = normalize_thunks(0, 0, ucs0, den_t0)
        pendq.append(thunks)

        # ---- remaining attention; fillers inside each ACT-bound block are:
        # the previous block's normalize chain + the next block's
        # projections (+ the qc0 half of the output projection during the
        # last block) ----
        blocks = [(0, qc) for qc in range(1, NQC)]
        blocks += [(j, qc) for j in range(1, DT) for qc in range(NQC)]
        owed = {blk: [] for blk in blocks}
        for (j, qc) in blocks:
            if (j, qc) != (0, 1):
                owed[(j, qc)].append(lambda j=j, qc=qc: qproj(j, qc))
            if qc == 0 and j >= 1:
                for n in range(NCH):
                    owed[(j, qc)].append(lambda j=j, n=n: kproj(j, 4 * n, 4))
        for f in owed[blocks[0]]:
            f()
        for bi, (j, qc) in enumerate(blocks):
            # projection fillers first; normalize chains run with a 2-block
            # lag so their slow DVE reciprocals never sit near a block
            # boundary (where they would delay the relus feeding the next
            # pair's attention)
            fillers = []
            if bi + 1 < len(blocks):
                fillers += owed[blocks[bi + 1]]
            last = bi == len(blocks) - 1
            if last:
                # flush remaining normalize chains, then gate + emit the qc0
                # half of the output projection so it runs inside this block
                while pendq:
                    fillers += pendq.pop(0)
                if NQC > 1:
                    fillers += [lambda: gate_outproj((DT - 1, 0))]
                    fillers += [lambda qt=qt: outproj(qt)
                                for qt in range(SQ_T // NQC)]
            elif len(pendq) >= 2:
                fillers += pendq.pop(0)
                if bi == len(blocks) - 2 and pendq:
                    fillers += pendq.pop(0)
            psU = new_psU()
            if (j, qc) == (0, 1) and store01:
                thunks, ucs = attn_span(
                    j, qc, list(range(len(store01), NKTP)), psU,
                    fillers, precomputed=store01)
            else:
                thunks, ucs = attn_span(j, qc, list(range(NKTP)), psU,
                                        fillers)
            pendq.append(thunks)
            last_ucs = ucs

        # ---- tail: last block's normalize + remaining output rows ----
        # Two of the final outproj chains are gated only on the last block's
        # PSUM copies (their bias + pairs-0..2 matmuls need nothing newer),
        # so the PE does useful work during the slow reciprocal chain and
        # stays HAM-warm; their pair-3 matmul still waits on the real OT
        # write.  Gate writes go on DVE BEFORE the normalize thunks so they
        # are not queued behind the reciprocals.
        qt_lo = SQ_T // NQC if NQC > 1 else 0
        early = []
        open_psO = []
        if NQC > 1 and last_ucs is not None:
            early = [qt_lo, qt_lo + 1]
            for qt, uc in zip(early, last_ucs):
                nc.vector.tensor_scalar(
                    OT1[:, qt * P:(qt + 1) * P],
                    OT1[:, qt * P:(qt + 1) * P],
                    uc[DH:DH + 1, 0:1], None, op0=ALU.min)
            # partial chains (bias + pairs 0..2): no pair-3 matmul yet, so
            # the in-order PE runs all 8 matmuls during the reciprocals
            # instead of stalling at the first chain's pair-3 wait
            for qt in early:
                psO = psum.tile([P, D], F32, tag="proj", name="psO")
                nc.tensor.matmul(psO, OT1[:, qt * P:(qt + 1) * P],
                                 b_row["bo"], start=True, stop=False)
                for j in range(DT - 1):
                    nc.tensor.matmul(psO, OT[:, j, qt * P:(qt + 1) * P],
                                     w_bf["wo"][:, j, :],
                                     start=False, stop=False)
                open_psO.append((qt, psO))
        while pendq:
            for f in pendq.pop(0):
                f()
        for qt, psO in open_psO:
            nc.tensor.matmul(psO, OT[:, DT - 1, qt * P:(qt + 1) * P],
                             w_bf["wo"][:, DT - 1, :],
                             start=False, stop=True)
            o_sb = work.tile([P, D], F32, tag="osb", bufs=2, name="o_sb")
            if qt % 2 == 0:
                nc.vector.tensor_scalar_max(o_sb, psO, 0.0)
            else:
                nc.scalar.activation(o_sb, psO, AF.Relu)
            nc.sync.dma_start(out[qt * P:(qt + 1) * P, :], o_sb)
        gate_outproj(blocks[-1])
        for qt in range(qt_lo, SQ_T):
            if qt not in early:
                outproj(qt)


_NC_CACHE = {}


def _get_nc(sk=S, sq=SQ_FULL, skip_vbias=False):
    key = (sk, sq, skip_vbias)
    if key not in _NC_CACHE:
        _NC_CACHE[key] = build_mha(sk, sq, skip_vbias)
    return _NC_CACHE[key]


def _tile_rows(a):
    """[D, n] -> SBUF layout [P, DT*n]: partition p gets rows p, 128+p, ..."""
    Dd, n = a.shape
    t = Dd // P
    return np.ascontiguousarray(
        a.reshape(t, P, n).transpose(1, 0, 2).reshape(P, t * n))


def _tile_rows_j(a):
    """[D, D] -> SBUF layout [P, DT(j)*DT(t)*P]: output-tile-major so each
    128-col output slice (one head pair's weights) is one contiguous DMA."""
    Dd, n = a.shape
    t, nj = Dd // P, n // P
    return np.ascontiguousarray(
        a.reshape(t, P, nj, P).transpose(1, 2, 0, 3).reshape(P, -1))


def _tile_chunks(a, chp):
    """[D, sk] -> chunk-major SBUF layout [P, NCH*DT*chp]: per partition,
    sequence chunks outermost so each chunk is one contiguous linear DMA."""
    Dd, sk = a.shape
    t, nch = Dd // P, sk // chp
    return np.ascontiguousarray(
        a.reshape(t, P, nch, chp).transpose(1, 2, 0, 3).reshape(P, -1))


def prep_inputs(x, Wq, bq, Wk, bk, Wv, bv, Wo, bo):
    """Host-side sharding/layout prep: bf16 casts, feature-major transpose,
    SBUF pre-tiling.  Returns the 8 per-core input maps."""
    bf = ml_dtypes.bfloat16
    f8 = ml_dtypes.float8_e4m3
    x = np.asarray(x, dtype=np.float32)
    shared = {
        "wq": _tile_rows_j(np.asarray(Wq, np.float32).astype(bf)),
        "wk": _tile_rows_j(np.asarray(Wk, np.float32).astype(bf)),
        "wv": _tile_rows(np.asarray(Wv, np.float32).astype(f8)),
        "wo": _tile_rows(np.asarray(Wo, np.float32).astype(bf)),
        "bq": np.ascontiguousarray(
            np.asarray(bq, np.float32).reshape(DT, P).T),
        "bk": np.ascontiguousarray(
            np.asarray(bk, np.float32).reshape(DT, P).T),
        "bv": np.asarray(bv, np.float32).astype(f8).reshape(1, D),
        "bo": np.asarray(bo, np.float32).astype(bf).reshape(1, D),
    }
    xT_b = [x[b].T.astype(bf) for b in range(B)]
    in_maps = []
    for c in range(NCORES):
        b, qo = divmod(c, QSPLIT)
        m = dict(shared)
        m["xT_bf"] = _tile_chunks(xT_b[b], P)
        m["xT_f8"] = _tile_chunks(xT_b[b].astype(f8), P)
        m["xqT_bf"] = _tile_rows(
            xT_b[b][:, qo * SQ_FULL:(qo + 1) * SQ_FULL])
        in_maps.append(m)
    return in_maps


def kernel(x, Wq, bq, Wk, bk, Wv, bv, Wo, bo, **run_kwargs):
    """Full-input entry point: shards across 8 NeuronCores, returns full out."""
    in_maps = prep_inputs(x, Wq, bq, Wk, bk, Wv, bv, Wo, bo)
    nc = _get_nc(skip_vbias=bool(np.all(np.asarray(bv) == 0)))
    res = bass_utils.run_bass_kernel_spmd(
        nc, in_maps, core_ids=list(range(NCORES)), **run_kwargs)
    full = np.empty((B, S, D), np.float32)
    for c in range(NCORES):
        b, qo = divmod(c, QSPLIT)
        full[b, qo * SQ_FULL:(qo + 1) * SQ_FULL] = res.results[c]["out"]
    if run_kwargs:
        return full, res
    return full

